# revision 5
# baseline (speedup 1.0000x reference)
"""SINDy-autoencoder forward+JVP kernel for 8 trn2 NeuronCores.

Network (widths 16384 -> 4096 -> 1024 -> 16, sigmoid; decoder mirrored):
  z   = enc(x)
  dz  = J_enc(x) @ dx          (forward-mode JVP through encoder)
  dzb = E_w @ sindy_library(z) + E_b
  xb  = dec(z)
  dxb = J_dec(z) @ dzb         (JVP through decoder)

Sharding (tensor-parallel, hardcoded for 8 cores):
  we_w0 [4096,16384]  row-sharded 512/core   (stage A)
  we_w1 [1024,4096]   col-sharded 512/core   (stage B, partial -> AllReduce)
  we_w2 [16,1024]     replicated             (stage C)
  SINDy/latent        replicated
  wd_w0 [1024,16]     replicated             (stage L0)
  wd_w1 [4096,1024]   row-sharded 512/core   (stage L1, shards -> AllGather)
  wd_w2 [16384,4096]  row-sharded 2048/core  (stage E, output shards)

Forward and JVP share every weight load: each matvec is a matmul with a
2-column moving operand [fwd, jvp].  All activation vectors live in SBUF as
[128-partition folds, (j, chunk)] column layouts so no transposes are needed
between stages.
"""

import numpy as np

NCORES = 8
IN, H1, H2, L = 16384, 4096, 1024, 16
H1S = H1 // NCORES      # 512  (encoder w0 rows / decoder wd1 rows per core)
OUTS = IN // NCORES     # 2048 (decoder wd2 rows per core)

# dtype for the two big GEMV stages (A and E) + mid stages.
# "float32"  = exact (PE does multi-pass fp32)
# "float32r" = fp32 data truncated to ~fp22 in the PE; ~2x faster weight path
BIG_DT_NAME = "float32"


# ----------------------------------------------------------------- program

_CACHE = {}


def _build_program():
    import concourse.bacc as bacc
    import concourse.mybir as mybir
    import concourse.tile as tile

    f32 = mybir.dt.float32
    big = getattr(mybir.dt, BIG_DT_NAME)
    AF = mybir.ActivationFunctionType
    OP = mybir.AluOpType

    nc = bacc.Bacc("TRN2", debug=False, target_bir_lowering=False,
                   num_devices=NCORES)

    # ---- I/O ----
    d_vin = nc.dram_tensor("vin", [128, 256], f32, kind="ExternalInput")
    d_w0 = nc.dram_tensor("w0t", [32, 4, 128, 512], big, kind="ExternalInput")
    d_w1 = nc.dram_tensor("w1t", [4, 128, 1024], big, kind="ExternalInput")
    d_w2 = nc.dram_tensor("w2t", [8, 128, 16], f32, kind="ExternalInput")
    d_wd0 = nc.dram_tensor("wd0t", [16, 1024], f32, kind="ExternalInput")
    d_wd1 = nc.dram_tensor("wd1t", [8, 128, 512], big, kind="ExternalInput")
    d_wd2 = nc.dram_tensor("wd2t", [32, 128, 2048], big, kind="ExternalInput")
    d_b0 = nc.dram_tensor("b0f", [128, 4], f32, kind="ExternalInput")
    d_b1 = nc.dram_tensor("b1f", [128, 8], f32, kind="ExternalInput")
    d_b2 = nc.dram_tensor("b2c", [16, 1], f32, kind="ExternalInput")
    d_bd0 = nc.dram_tensor("bd0f", [128, 8], f32, kind="ExternalInput")
    d_bd1 = nc.dram_tensor("bd1f", [128, 4], f32, kind="ExternalInput")
    d_bd2 = nc.dram_tensor("bd2f", [128, 16], f32, kind="ExternalInput")
    d_i16 = nc.dram_tensor("i16", [16, 16], f32, kind="ExternalInput")
    d_r1 = nc.dram_tensor("r1", [16, 128], f32, kind="ExternalInput")
    d_r2a = nc.dram_tensor("r2a", [16, 128], f32, kind="ExternalInput")
    d_r2b = nc.dram_tensor("r2b", [16, 128], f32, kind="ExternalInput")
    d_c3 = nc.dram_tensor("c3stack", [128, 512], f32, kind="ExternalInput")
    d_c2 = nc.dram_tensor("c2stack", [16, 256], f32, kind="ExternalInput")
    d_ewl = nc.dram_tensor("ewlint", [16, 16], f32, kind="ExternalInput")
    d_c0 = nc.dram_tensor("c0", [16, 1], f32, kind="ExternalInput")

    d_xb = nc.dram_tensor("xb_sh", [OUTS], f32, kind="ExternalOutput")
    d_dxb = nc.dram_tensor("dxb_sh", [OUTS], f32, kind="ExternalOutput")
    d_lat = nc.dram_tensor("lat", [16, 3], f32, kind="ExternalOutput")

    RG = [list(range(NCORES))]

    with tile.TileContext(nc) as tc:
        with (
            tc.tile_pool(name="consts", bufs=1) as cp,
            tc.tile_pool(name="w0s", bufs=3) as w0p,
            tc.tile_pool(name="wd2s", bufs=12) as wd2p,
            tc.tile_pool(name="act", bufs=1) as ap,
            tc.tile_pool(name="ps", bufs=2, space="PSUM") as pp,
            tc.tile_pool(name="dram", bufs=1, space="DRAM") as dp,
        ):
            # ---------- constant loads ----------
            vin = cp.tile([128, 256], f32, tag="vin")
            nc.sync.dma_start(vin[:, :], d_vin[:, :])
            w1s = cp.tile([128, 4096], big, tag="w1s")
            nc.sync.dma_start(
                w1s[:, :], d_w1.ap().rearrange("k p m -> p k m"))
            w2s = cp.tile([128, 128], f32, tag="w2s")
            nc.sync.dma_start(
                w2s[:, :], d_w2.ap().rearrange("k p m -> p k m"))
            wd0s = cp.tile([16, 1024], f32, tag="wd0s")
            nc.sync.dma_start(wd0s[:, :], d_wd0[:, :])
            wd1s = cp.tile([128, 4096], big, tag="wd1s")
            nc.sync.dma_start(
                wd1s[:, :], d_wd1.ap().rearrange("k p m -> p k m"))

            def cload(dram, shape, tag, dt=f32):
                t = cp.tile(shape, dt, tag=tag)
                nc.sync.dma_start(t[:, :], dram[:, :])
                return t

            b0f = cload(d_b0, [128, 4], "b0f")
            b1f = cload(d_b1, [128, 8], "b1f")
            b2c = cload(d_b2, [16, 1], "b2c")
            bd0f = cload(d_bd0, [128, 8], "bd0f")
            bd1f = cload(d_bd1, [128, 4], "bd1f")
            bd2f = cload(d_bd2, [128, 16], "bd2f")
            i16 = cload(d_i16, [16, 16], "i16")
            r1 = cload(d_r1, [16, 128], "r1")
            r2a = cload(d_r2a, [16, 128], "r2a")
            r2b = cload(d_r2b, [16, 128], "r2b")
            c3s = cload(d_c3, [128, 512], "c3s")
            c2s = cload(d_c2, [16, 256], "c2s")
            ewl = cload(d_ewl, [16, 16], "ewl")
            c0s = cload(d_c0, [16, 1], "c0s")

            def jview(t, j=2):
                # [P, (j c)] -> [P, c, j]: [:, c, :] = the (fwd, jvp) col pair
                return t.rearrange("p (j c) -> p c j", j=j)

            def pview(t):
                # psum pair-major [P, (m j)] -> [P, m, j]
                return t.rearrange("p (m j) -> p m j", j=2)

            vin_r = jview(vin)
            vin_big = jview(vin.bitcast(big)) if big is not f32 else vin_r

            # ---------- stage A: a0 = W0s @ [x dx]  (512 rows/core) ----------
            pa = pp.tile([128, 8], f32, tag="big")
            pa_r = pview(pa)
            for g in range(32):            # 32 slabs x 4 k-chunks x 1MB
                slab = w0p.tile([128, 2048], big, tag="w0slab")
                nc.sync.dma_start(
                    slab[:, :], d_w0[g].rearrange("k p m -> p k m"))
                for ks in range(4):
                    kc = g * 4 + ks
                    rhs = vin_big[:, kc, :]
                    for mt in range(4):
                        nc.tensor.matmul(
                            pa[:, mt * 2:(mt + 1) * 2],
                            lhsT=slab[:, ks * 512 + mt * 128:
                                      ks * 512 + (mt + 1) * 128],
                            rhs=rhs,
                            start=(kc == 0 and mt == 0),
                            stop=(kc == 127 and mt == 3))

            # evacuate: s0 = sigmoid(a0 + b0), u0' = s0(1-s0)*u0
            s0u0 = ap.tile([128, 8], f32, tag="s0u0")
            tmp0 = ap.tile([128, 4], f32, tag="tmp0")
            nc.vector.tensor_tensor(
                out=tmp0[:, :], in0=pa_r[:, :, 0], in1=b0f[:, :], op=OP.add)
            nc.scalar.activation(s0u0[:, 0:4], tmp0[:, :], AF.Sigmoid)
            d0 = ap.tile([128, 4], f32, tag="d0")
            nc.vector.tensor_scalar(
                out=d0[:, :], in0=s0u0[:, 0:4], scalar1=-1.0, scalar2=1.0,
                op0=OP.mult, op1=OP.add)
            nc.vector.tensor_tensor(
                out=d0[:, :], in0=d0[:, :], in1=s0u0[:, 0:4], op=OP.mult)
            nc.vector.tensor_tensor(
                out=s0u0[:, 4:8], in0=d0[:, :], in1=pa_r[:, :, 1], op=OP.mult)
            s0u0_r = jview(s0u0)
            s0u0_big = jview(s0u0.bitcast(big)) if big is not f32 else s0u0_r

            # ---------- stage B: a1_partial = W1s @ [s0 u0']  ----------
            pb = pp.tile([128, 16], f32, tag="big")
            pb_r = pview(pb)
            for kc in range(4):
                rhs = s0u0_big[:, kc, :]
                for mt in range(8):
                    nc.tensor.matmul(
                        pb[:, mt * 2:(mt + 1) * 2],
                        lhsT=w1s[:, kc * 1024 + mt * 128:
                                 kc * 1024 + (mt + 1) * 128],
                        rhs=rhs,
                        start=(kc == 0 and mt == 0),
                        stop=(kc == 3 and mt == 7))

            # ---------- AllReduce of [a1; u1] partials ----------
            arst = ap.tile([128, 16], f32, tag="arst")
            nc.vector.tensor_copy(arst[:, :], pb[:, :])
            ar_in = dp.tile([128, 16], f32, tag="ar_in")
            ar_out = dp.tile([128, 16], f32, tag="ar_out")
            nc.sync.dma_start(ar_in[:, :], arst[:, :])
            nc.gpsimd.collective_compute(
                "AllReduce", OP.add, replica_groups=RG,
                ins=[ar_in.opt()], outs=[ar_out.opt()])
            a1f = ap.tile([128, 16], f32, tag="a1f")
            nc.sync.dma_start(a1f[:, :], ar_out[:, :])

            # s1 = sigmoid(a1 + b1), u1' = s1(1-s1)*u1
            s1u1 = ap.tile([128, 16], f32, tag="s1u1")
            tmp1 = ap.tile([128, 8], f32, tag="tmp1")
            a1f_r = pview(a1f)
            nc.vector.tensor_tensor(
                out=tmp1[:, :], in0=a1f_r[:, :, 0], in1=b1f[:, :], op=OP.add)
            nc.scalar.activation(s1u1[:, 0:8], tmp1[:, :], AF.Sigmoid)
            d1 = ap.tile([128, 8], f32, tag="d1")
            nc.vector.tensor_scalar(
                out=d1[:, :], in0=s1u1[:, 0:8], scalar1=-1.0, scalar2=1.0,
                op0=OP.mult, op1=OP.add)
            nc.vector.tensor_tensor(
                out=d1[:, :], in0=d1[:, :], in1=s1u1[:, 0:8], op=OP.mult)
            nc.vector.tensor_tensor(
                out=s1u1[:, 8:16], in0=d1[:, :], in1=a1f_r[:, :, 1], op=OP.mult)
            s1u1_r = jview(s1u1)

            # ---------- stage C: [z_pre dz] = W2 @ [s1 u1'] ----------
            pc = pp.tile([16, 2], f32, tag="lat")
            for kc in range(8):
                nc.tensor.matmul(
                    pc[:, :], lhsT=w2s[:, kc * 16:(kc + 1) * 16],
                    rhs=s1u1_r[:, kc, :],
                    start=(kc == 0), stop=(kc == 7))
            lat = ap.tile([16, 4], f32, tag="lat_sb")
            nc.vector.tensor_tensor(
                out=lat[:, 0:1], in0=pc[:, 0:1], in1=b2c[:, :], op=OP.add)
            nc.vector.tensor_copy(lat[:, 1:2], pc[:, 1:2])
            zcol = lat[:, 0:1]

            # ---------- SINDy latent: dzb = E_w @ theta(z) + E_b ----------
            # z as a row [1,16]
            p_zr = pp.tile([1, 16], f32, tag="lat")
            nc.tensor.matmul(p_zr[:, :], lhsT=zcol, rhs=i16[:, :],
                             start=True, stop=True)
            zrow = ap.tile([1, 16], f32, tag="zrow")
            nc.vector.tensor_copy(zrow[:, :], p_zr[:, :])
            # ZZ[a,b] = z_a z_b
            p_zz = pp.tile([16, 16], f32, tag="lat")
            nc.tensor.matmul(p_zz[:, :], lhsT=zrow[:, :], rhs=zrow[:, :],
                             start=True, stop=True)
            zz = ap.tile([16, 16], f32, tag="zz")
            nc.vector.tensor_copy(zz[:, :], p_zz[:, :])
            # A2[p, jlo*16+k] = ZZ[2*(p%8)+jlo, k]
            p_a2 = pp.tile([128, 32], f32, tag="lat")
            nc.tensor.matmul(p_a2[:, 0:16], lhsT=r2a[:, :], rhs=zz[:, :],
                             start=True, stop=False)
            nc.tensor.matmul(p_a2[:, 16:32], lhsT=r2b[:, :], rhs=zz[:, :],
                             start=False, stop=True)
            a2 = ap.tile([128, 32], f32, tag="a2")
            nc.vector.tensor_copy(a2[:, :], p_a2[:, :])
            # zrep[p] = z_{p//8}
            p_zp = pp.tile([128, 1], f32, tag="lat")
            nc.tensor.matmul(p_zp[:, :], lhsT=r1[:, :], rhs=zcol,
                             start=True, stop=True)
            zrep = ap.tile([128, 1], f32, tag="zrep")
            nc.vector.tensor_copy(zrep[:, :], p_zp[:, :])
            # P3v[p,f] = z_i z_j z_k  (i=p//8, j=2*(p%8)+f//16, k=f%16)
            p3v = ap.tile([128, 32], f32, tag="p3v")
            nc.vector.tensor_scalar(
                out=p3v[:, :], in0=a2[:, :], scalar1=zrep[:, 0:1],
                scalar2=None, op0=OP.mult)
            # accumulate linear + quadratic + cubic contributions
            p_dzb = pp.tile([16, 1], f32, tag="lat")
            nc.tensor.matmul(p_dzb[:, :], lhsT=ewl[:, :], rhs=zcol,
                             start=True, stop=False)
            for b in range(16):
                nc.tensor.matmul(
                    p_dzb[:, :], lhsT=c2s[:, b * 16:(b + 1) * 16],
                    rhs=zz[:, b:b + 1], start=False, stop=False)
            for f in range(32):
                nc.tensor.matmul(
                    p_dzb[:, :], lhsT=c3s[:, f * 16:(f + 1) * 16],
                    rhs=p3v[:, f:f + 1], start=False, stop=(f == 31))
            nc.vector.tensor_tensor(
                out=lat[:, 2:3], in0=p_dzb[:, :], in1=c0s[:, :], op=OP.add)

            # ---------- stage L0: [t0_pre v0] = wd0 @ [z dzb] ----------
            zdzb = ap.tile([16, 2], f32, tag="zdzb")
            nc.vector.tensor_copy(zdzb[:, 0:1], lat[:, 0:1])
            nc.vector.tensor_copy(zdzb[:, 1:2], lat[:, 2:3])
            pl0 = pp.tile([128, 16], f32, tag="big")
            pl0_r = pview(pl0)
            for mt in range(8):
                nc.tensor.matmul(
                    pl0[:, mt * 2:(mt + 1) * 2],
                    lhsT=wd0s[:, mt * 128:(mt + 1) * 128], rhs=zdzb[:, :],
                    start=(mt == 0), stop=(mt == 7))
            t0v0 = ap.tile([128, 16], f32, tag="t0v0")
            tmp2 = ap.tile([128, 8], f32, tag="tmp2")
            nc.vector.tensor_tensor(
                out=tmp2[:, :], in0=pl0_r[:, :, 0], in1=bd0f[:, :], op=OP.add)
            nc.scalar.activation(t0v0[:, 0:8], tmp2[:, :], AF.Sigmoid)
            d2 = ap.tile([128, 8], f32, tag="d2")
            nc.vector.tensor_scalar(
                out=d2[:, :], in0=t0v0[:, 0:8], scalar1=-1.0, scalar2=1.0,
                op0=OP.mult, op1=OP.add)
            nc.vector.tensor_tensor(
                out=d2[:, :], in0=d2[:, :], in1=t0v0[:, 0:8], op=OP.mult)
            nc.vector.tensor_tensor(
                out=t0v0[:, 8:16], in0=d2[:, :], in1=pl0_r[:, :, 1], op=OP.mult)
            t0v0_r = jview(t0v0)
            t0v0_big = jview(t0v0.bitcast(big)) if big is not f32 else t0v0_r

            # ---------- stage L1: wd1 shard @ [t0 v0'] (512 rows/core) -----
            pl1 = pp.tile([128, 8], f32, tag="big")
            pl1_r = pview(pl1)
            for kc in range(8):
                rhs = t0v0_big[:, kc, :]
                for mt in range(4):
                    nc.tensor.matmul(
                        pl1[:, mt * 2:(mt + 1) * 2],
                        lhsT=wd1s[:, kc * 512 + mt * 128:
                                  kc * 512 + (mt + 1) * 128],
                        rhs=rhs,
                        start=(kc == 0 and mt == 0),
                        stop=(kc == 7 and mt == 3))
            tv = ap.tile([128, 8], f32, tag="tv")
            tmp3 = ap.tile([128, 4], f32, tag="tmp3")
            nc.vector.tensor_tensor(
                out=tmp3[:, :], in0=pl1_r[:, :, 0], in1=bd1f[:, :], op=OP.add)
            nc.scalar.activation(tv[:, 0:4], tmp3[:, :], AF.Sigmoid)
            d3 = ap.tile([128, 4], f32, tag="d3")
            nc.vector.tensor_scalar(
                out=d3[:, :], in0=tv[:, 0:4], scalar1=-1.0, scalar2=1.0,
                op0=OP.mult, op1=OP.add)
            nc.vector.tensor_tensor(
                out=d3[:, :], in0=d3[:, :], in1=tv[:, 0:4], op=OP.mult)
            nc.vector.tensor_tensor(
                out=tv[:, 4:8], in0=d3[:, :], in1=pl1_r[:, :, 1], op=OP.mult)

            # ---------- AllGather [t1 v1'] shards ----------
            ag_in = dp.tile([128, 8], f32, tag="ag_in")
            ag_out = dp.tile([1024, 8], f32, tag="ag_out")
            nc.sync.dma_start(ag_in[:, :], tv[:, :])
            nc.gpsimd.collective_compute(
                "AllGather", OP.bypass, replica_groups=RG,
                ins=[ag_in.opt()], outs=[ag_out.opt()])
            t1full = cp.tile([128, 64], f32, tag="t1full")
            nc.sync.dma_start(
                t1full[:, :], ag_out.rearrange("(r p) q -> p r q", p=128))
            t1f = t1full.bitcast(big) if big is not f32 else t1full
            t1f_r = t1f.rearrange("p (r j m) -> p r m j", r=8, j=2, m=4)

            # ---------- stage E: [xb dxb] = wd2 shard @ [t1 v1'] ----------
            pe = pp.tile([128, 32], f32, tag="big")
            pe_r = pview(pe)
            for kc in range(32):
                slab = wd2p.tile([128, 2048], big, tag="wd2slab")
                nc.sync.dma_start(slab[:, :], d_wd2[kc])
                rhs = t1f_r[:, kc // 4, kc % 4, :]
                for mt in range(16):
                    nc.tensor.matmul(
                        pe[:, mt * 2:(mt + 1) * 2],
                        lhsT=slab[:, mt * 128:(mt + 1) * 128],
                        rhs=rhs,
                        start=(kc == 0 and mt == 0),
                        stop=(kc == 31 and mt == 15))

            xbs = ap.tile([128, 16], f32, tag="xbs")
            nc.vector.tensor_tensor(
                out=xbs[:, :], in0=pe_r[:, :, 0], in1=bd2f[:, :], op=OP.add)
            dxbs = ap.tile([128, 16], f32, tag="dxbs")
            nc.vector.tensor_copy(dxbs[:, :], pe_r[:, :, 1])

            nc.sync.dma_start(
                d_xb.ap().rearrange("(m p) -> p m", p=128), xbs[:, :])
            nc.sync.dma_start(
                d_dxb.ap().rearrange("(m p) -> p m", p=128), dxbs[:, :])
            nc.sync.dma_start(d_lat[:, :], lat[:, 0:3])

    nc.compile()
    return nc


def get_program():
    if "nc" not in _CACHE:
        _CACHE["nc"] = _build_program()
    return _CACHE["nc"]


# ------------------------------------------------------------- host prep

def prepare_in_maps(inputs):
    """Shard + lay out the full inputs into 8 per-core input maps."""
    f = np.float32
    x = np.asarray(inputs["x"], f)
    dx = np.asarray(inputs["dx"], f)
    E_w = np.asarray(inputs["E_w"], f)
    E_b = np.asarray(inputs["E_b"], f)
    we_w0 = np.asarray(inputs["we_w0"], f)
    we_b0 = np.asarray(inputs["we_b0"], f)
    we_w1 = np.asarray(inputs["we_w1"], f)
    we_b1 = np.asarray(inputs["we_b1"], f)
    we_w2 = np.asarray(inputs["we_w2"], f)
    we_b2 = np.asarray(inputs["we_b2"], f)
    wd_w0 = np.asarray(inputs["wd_w0"], f)
    wd_b0 = np.asarray(inputs["wd_b0"], f)
    wd_w1 = np.asarray(inputs["wd_w1"], f)
    wd_b1 = np.asarray(inputs["wd_b1"], f)
    wd_w2 = np.asarray(inputs["wd_w2"], f)
    wd_b2 = np.asarray(inputs["wd_b2"], f)

    # vin[p, j*128+c] = (x, dx)[c*128 + p]
    vin = np.stack([x.reshape(128, 128).T, dx.reshape(128, 128).T], axis=0)
    vin = np.ascontiguousarray(vin.transpose(1, 0, 2).reshape(128, 256))

    # SINDy coefficient folding (replicated)
    c0 = (E_b + E_w[:, 0:L].sum(axis=1)).reshape(16, 1).astype(f)
    ewlint = np.ascontiguousarray(E_w[:, L:2 * L].T)
    c2stack = np.zeros((16, 256), f)
    q = 2 * L
    for i in range(L):
        for j in range(i, L):
            c2stack[i, j * 16:(j + 1) * 16] = E_w[:, q]
            q += 1
    c3stack = np.zeros((128, 512), f)
    for i in range(L):
        for j in range(i, L):
            for k in range(j, L):
                p = i * 8 + j // 2
                fidx = (j % 2) * 16 + k
                c3stack[p, fidx * 16:(fidx + 1) * 16] += E_w[:, q]
                q += 1
    assert q == E_w.shape[1]

    r1 = np.zeros((16, 128), f)
    r1[np.arange(128) // 8, np.arange(128)] = 1.0
    r2a = np.zeros((16, 128), f)
    r2a[2 * (np.arange(128) % 8), np.arange(128)] = 1.0
    r2b = np.zeros((16, 128), f)
    r2b[2 * (np.arange(128) % 8) + 1, np.arange(128)] = 1.0

    shared = {
        "vin": vin,
        "w1t": None,  # per-core below
        "w2t": np.ascontiguousarray(we_w2.T).reshape(8, 128, 16),
        "wd0t": np.ascontiguousarray(wd_w0.T),
        "b1f": np.ascontiguousarray(we_b1.reshape(8, 128).T),
        "b2c": we_b2.reshape(16, 1),
        "bd0f": np.ascontiguousarray(wd_b0.reshape(8, 128).T),
        "i16": np.eye(16, dtype=f),
        "r1": r1, "r2a": r2a, "r2b": r2b,
        "c3stack": c3stack, "c2stack": c2stack,
        "ewlint": ewlint, "c0": c0,
    }
    del shared["w1t"]

    in_maps = []
    for c in range(NCORES):
        r0 = slice(c * H1S, (c + 1) * H1S)
        r2 = slice(c * OUTS, (c + 1) * OUTS)
        m = dict(shared)
        m["w0t"] = np.ascontiguousarray(we_w0[r0, :].T).reshape(
            32, 4, 128, 512)
        m["w1t"] = np.ascontiguousarray(we_w1[:, r0].T).reshape(4, 128, 1024)
        m["wd1t"] = np.ascontiguousarray(wd_w1[r0, :].T).reshape(8, 128, 512)
        m["wd2t"] = np.ascontiguousarray(wd_w2[r2, :].T).reshape(
            32, 128, 2048)
        m["b0f"] = np.ascontiguousarray(we_b0[r0].reshape(4, 128).T)
        m["bd1f"] = np.ascontiguousarray(wd_b1[r0].reshape(4, 128).T)
        m["bd2f"] = np.ascontiguousarray(wd_b2[r2].reshape(16, 128).T)
        in_maps.append(m)
    return in_maps


def assemble_outputs(results):
    """results: list (per core) of {name: np.ndarray}."""
    lat = results[0]["lat"]
    z = np.ascontiguousarray(lat[:, 0])
    dz = np.ascontiguousarray(lat[:, 1])
    dzb = np.ascontiguousarray(lat[:, 2])
    xb = np.concatenate(
        [np.asarray(results[c]["xb_sh"]).reshape(-1) for c in range(NCORES)])
    dxb = np.concatenate(
        [np.asarray(results[c]["dxb_sh"]).reshape(-1) for c in range(NCORES)])
    return (z, dz, dzb, xb, dxb)


def kernel(**inputs):
    from concourse.bass_utils import run_bass_kernel_spmd
    nc = get_program()
    in_maps = prepare_in_maps(inputs)
    res = run_bass_kernel_spmd(nc, in_maps, core_ids=list(range(NCORES)))
    return assemble_outputs(res.results)


# revision 13
# speedup vs baseline: 1.5815x; 1.5815x over previous
"""SINDy-autoencoder forward+JVP kernel for 8 trn2 NeuronCores.

Network (widths 16384 -> 4096 -> 1024 -> 16, sigmoid; decoder mirrored):
  z   = enc(x)
  dz  = J_enc(x) @ dx          (forward-mode JVP through encoder)
  dzb = E_w @ sindy_library(z) + E_b
  xb  = dec(z)
  dxb = J_dec(z) @ dzb         (JVP through decoder)

Sharding (tensor-parallel, hardcoded for 8 cores):
  we_w0 [4096,16384]  row-sharded 512/core   (stage A, weight-moving)
  we_w1 [1024,4096]   row-sharded 128/core   (stage B; AllGather before+after)
  we_w2 [16,1024]     replicated             (stage C)
  SINDy/latent        replicated
  wd_w0 [1024,16]     replicated             (stage L0)
  wd_w1 [4096,1024]   row-sharded 512/core   (stage L1; AllGather after)
  wd_w2 [16384,4096]  row-sharded 2048/core  (stage E, weight-moving)

The two big stages (A, E) stream the weights as the PE's *moving* operand
(N=512 columns per matmul) against a tiny stationary holding the
[fwd, jvp] activation pair, so the PE consumes 128 weights/cycle instead
of being weight-load bound.  fp32 accuracy is kept by splitting both the
weights and the activations into bf16 hi+lo parts (bf16 upcasts exactly
into the PE's fp22 multiply path) and accumulating the three significant
cross terms (hi*Whi + lo*Whi + hi*Wlo) in fp32 PSUM.

Forward and JVP share every weight byte: each matvec is a matmul with a
2-column stationary pair [fwd, jvp].
"""

import numpy as np
import ml_dtypes

BF16 = ml_dtypes.bfloat16

NCORES = 8
IN, H1, H2, L = 16384, 4096, 1024, 16
H1S = H1 // NCORES      # 512  (encoder w0 rows / decoder wd1 rows per core)
H2S = H2 // NCORES      # 128  (encoder w1 rows per core)
OUTS = IN // NCORES     # 2048 (decoder wd2 rows per core)

WD2_BUFS = 12           # wd2 slab prefetch ring (1MB each)

_CACHE = {}


def _build_program():
    import concourse.bacc as bacc
    import concourse.mybir as mybir
    import concourse.tile as tile

    f32 = mybir.dt.float32
    bf16 = mybir.dt.bfloat16
    AF = mybir.ActivationFunctionType
    OP = mybir.AluOpType

    nc = bacc.Bacc("TRN2", debug=False, target_bir_lowering=False,
                   num_devices=NCORES)

    # ---- I/O ----
    d_vins = nc.dram_tensor("vins", [128, 512], bf16, kind="ExternalInput")
    d_w0 = nc.dram_tensor("w0t", [32, 4, 2, 128, 512], bf16,
                          kind="ExternalInput")
    d_w1 = nc.dram_tensor("w1t", [32, 128, 128], f32, kind="ExternalInput")
    d_w2 = nc.dram_tensor("w2t", [8, 128, 16], f32, kind="ExternalInput")
    d_wd0 = nc.dram_tensor("wd0t", [16, 1024], f32, kind="ExternalInput")
    d_wd1 = nc.dram_tensor("wd1t", [8, 128, 512], f32, kind="ExternalInput")
    d_wd2 = nc.dram_tensor("wd2t", [32, 2, 128, 2048], bf16,
                           kind="ExternalInput")
    d_b0 = nc.dram_tensor("b0row", [1, 512], f32, kind="ExternalInput")
    d_b1 = nc.dram_tensor("b1sh", [128, 1], f32, kind="ExternalInput")
    d_b2 = nc.dram_tensor("b2c", [16, 1], f32, kind="ExternalInput")
    d_bd0 = nc.dram_tensor("bd0f", [128, 8], f32, kind="ExternalInput")
    d_bd1 = nc.dram_tensor("bd1f", [128, 4], f32, kind="ExternalInput")
    d_bd2 = nc.dram_tensor("bd2row", [1, 2048], f32, kind="ExternalInput")
    d_i16 = nc.dram_tensor("i16", [16, 16], f32, kind="ExternalInput")
    d_r1 = nc.dram_tensor("r1", [16, 128], f32, kind="ExternalInput")
    d_r2a = nc.dram_tensor("r2a", [16, 128], f32, kind="ExternalInput")
    d_r2b = nc.dram_tensor("r2b", [16, 128], f32, kind="ExternalInput")
    d_c3 = nc.dram_tensor("c3stack", [128, 512], f32, kind="ExternalInput")
    d_c2 = nc.dram_tensor("c2stack", [16, 256], f32, kind="ExternalInput")
    d_ewl = nc.dram_tensor("ewlint", [16, 16], f32, kind="ExternalInput")
    d_c0 = nc.dram_tensor("c0", [16, 1], f32, kind="ExternalInput")

    d_xb = nc.dram_tensor("xb_sh", [OUTS], f32, kind="ExternalOutput")
    d_dxb = nc.dram_tensor("dxb_sh", [OUTS], f32, kind="ExternalOutput")
    d_lat = nc.dram_tensor("lat", [16, 3], f32, kind="ExternalOutput")

    RG = [list(range(NCORES))]

    with tile.TileContext(nc) as tc:
        with (
            tc.tile_pool(name="consts", bufs=1) as cp,
            tc.tile_pool(name="w0s", bufs=3) as w0p,
            tc.tile_pool(name="wd2s", bufs=WD2_BUFS) as wd2p,
            tc.tile_pool(name="act", bufs=1) as ap,
            tc.tile_pool(name="dram", bufs=1, space="DRAM") as dp,
        ):
            # ---------- constant loads ----------
            vins = cp.tile([128, 512], bf16, tag="vins")
            nc.sync.dma_start(vins[:, :], d_vins[:, :])
            w1s = cp.tile([128, 4096], f32, tag="w1s")
            nc.sync.dma_start(
                w1s[:, :], d_w1.ap().rearrange("k p m -> p k m"))
            w2s = cp.tile([128, 128], f32, tag="w2s")
            nc.sync.dma_start(
                w2s[:, :], d_w2.ap().rearrange("k p m -> p k m"))
            wd0s = cp.tile([16, 1024], f32, tag="wd0s")
            nc.sync.dma_start(wd0s[:, :], d_wd0[:, :])
            wd1s = cp.tile([128, 4096], f32, tag="wd1s")
            nc.sync.dma_start(
                wd1s[:, :], d_wd1.ap().rearrange("k p m -> p k m"))

            def cload(dram, shape, tag, dt=f32):
                t = cp.tile(shape, dt, tag=tag)
                nc.sync.dma_start(t[:, :], dram[:, :])
                return t

            b0row = cload(d_b0, [1, 512], "b0row")
            b1sh = cload(d_b1, [128, 1], "b1sh")
            b2c = cload(d_b2, [16, 1], "b2c")
            bd0f = cload(d_bd0, [128, 8], "bd0f")
            bd1f = cload(d_bd1, [128, 4], "bd1f")
            bd2row = cload(d_bd2, [1, 2048], "bd2row")
            i16 = cload(d_i16, [16, 16], "i16")
            r1 = cload(d_r1, [16, 128], "r1")
            r2a = cload(d_r2a, [16, 128], "r2a")
            r2b = cload(d_r2b, [16, 128], "r2b")
            c3s = cload(d_c3, [128, 512], "c3s")
            c2s = cload(d_c2, [16, 256], "c2s")
            ewl = cload(d_ewl, [16, 16], "ewl")
            c0s = cload(d_c0, [16, 1], "c0s")

            def pview(t):
                # psum pair-major [P, (m j)] -> [P, m, j]
                return t.rearrange("p (m j) -> p m j", j=2)

            with tc.tile_pool(name="ps1", bufs=1, space="PSUM") as pp:
                # ------- stage A: [a0; u0] = W0s @ [x dx] (weight-moving) ----
                pA = pp.tile([2, 512], f32, tag="row")
                for g in range(32):          # 32 slabs x (4 kc x 2 h) x 1MB
                    slab = w0p.tile([128, 4096], bf16, tag="w0slab")
                    nc.sync.dma_start(
                        slab[:, :], d_w0[g].rearrange("k h p m -> p k h m"))
                    for ks in range(4):
                        kc = g * 4 + ks
                        hi = vins[:, kc * 4:kc * 4 + 2]
                        lo = vins[:, kc * 4 + 2:kc * 4 + 4]
                        whi = slab[:, ks * 1024:ks * 1024 + 512]
                        wlo = slab[:, ks * 1024 + 512:ks * 1024 + 1024]
                        for i, (a, w) in enumerate(
                                [(hi, whi), (lo, whi), (hi, wlo)]):
                            nc.tensor.matmul(
                                pA[:, :], lhsT=a, rhs=w,
                                start=(kc == 0 and i == 0),
                                stop=(kc == 127 and i == 2))

                # s0 = sigmoid(a0 + b0), u0' = s0(1-s0)*u0   (row layout)
                rows2 = ap.tile([2, 512], f32, tag="rows2")
                nc.vector.tensor_copy(rows2[:, :], pA[:, :])
                u0raw = ap.tile([1, 512], f32, tag="u0raw")
                nc.sync.dma_start(u0raw[:, :], rows2[1:2, :])
                s0row = ap.tile([1, 512], f32, tag="s0row")
                tmpr = ap.tile([1, 512], f32, tag="tmpr")
                nc.vector.tensor_tensor(
                    out=tmpr[:, :], in0=rows2[0:1, :], in1=b0row[:, :],
                    op=OP.add)
                nc.scalar.activation(s0row[:, :], tmpr[:, :], AF.Sigmoid)
                d0r = ap.tile([1, 512], f32, tag="d0r")
                nc.vector.tensor_scalar(
                    out=d0r[:, :], in0=s0row[:, :], scalar1=-1.0,
                    scalar2=1.0, op0=OP.mult, op1=OP.add)
                nc.vector.tensor_tensor(
                    out=d0r[:, :], in0=d0r[:, :], in1=s0row[:, :],
                    op=OP.mult)
                u0p = ap.tile([1, 512], f32, tag="u0p")
                nc.vector.tensor_tensor(
                    out=u0p[:, :], in0=d0r[:, :], in1=u0raw[:, :],
                    op=OP.mult)

                # ------- AllGather 1: full [s0; u0'] ----------
                ag1_in = dp.tile([2, 512], f32, tag="ag1_in")
                ag1_out = dp.tile([16, 512], f32, tag="ag1_out")
                nc.sync.dma_start(ag1_in[0:1, :], s0row[:, :])
                nc.sync.dma_start(ag1_in[1:2, :], u0p[:, :])
                nc.gpsimd.collective_compute(
                    "AllGather", OP.bypass, replica_groups=RG,
                    ins=[ag1_in.opt()], outs=[ag1_out.opt()])
                agsb = ap.tile([16, 512], f32, tag="agsb")
                nc.sync.dma_start(agsb[:, :], ag1_out[:, :])
                # transpose to column layout: s0u0f[p, c*16 + r*2 + j]
                # (stage-B chunk kc = c*8 + r covers s0 indices r*512+c*128+p)
                s0u0f = ap.tile([128, 64], f32, tag="s0u0f")
                for c in range(4):
                    ptr = pp.tile([128, 16], f32, tag="lat", name=f"ptr{c}")
                    nc.tensor.transpose(
                        ptr[:, :], agsb[:, c * 128:(c + 1) * 128], i16[:, :])
                    nc.vector.tensor_copy(
                        s0u0f[:, c * 16:(c + 1) * 16], ptr[:, :])

                # ------- stage B: a1 shard = W1s(128 rows) @ [s0 u0'] -------
                pB = pp.tile([128, 2], f32, tag="big")
                for kc in range(32):
                    nc.tensor.matmul(
                        pB[:, :], lhsT=w1s[:, kc * 128:(kc + 1) * 128],
                        rhs=s0u0f[:, kc * 2:kc * 2 + 2],
                        start=(kc == 0), stop=(kc == 31))
                s1u1sh = ap.tile([128, 2], f32, tag="s1u1sh")
                tmps = ap.tile([128, 1], f32, tag="tmps")
                nc.vector.tensor_tensor(
                    out=tmps[:, :], in0=pB[:, 0:1], in1=b1sh[:, :], op=OP.add)
                nc.scalar.activation(s1u1sh[:, 0:1], tmps[:, :], AF.Sigmoid)
                d1s = ap.tile([128, 1], f32, tag="d1s")
                nc.vector.tensor_scalar(
                    out=d1s[:, :], in0=s1u1sh[:, 0:1], scalar1=-1.0,
                    scalar2=1.0, op0=OP.mult, op1=OP.add)
                nc.vector.tensor_tensor(
                    out=d1s[:, :], in0=d1s[:, :], in1=s1u1sh[:, 0:1],
                    op=OP.mult)
                nc.vector.tensor_tensor(
                    out=s1u1sh[:, 1:2], in0=d1s[:, :], in1=pB[:, 1:2],
                    op=OP.mult)

                # ------- AllGather 2: full [s1; u1'] ----------
                ag2_in = dp.tile([128, 2], f32, tag="ag2_in")
                ag2_out = dp.tile([1024, 2], f32, tag="ag2_out")
                nc.sync.dma_start(ag2_in[:, :], s1u1sh[:, :])
                nc.gpsimd.collective_compute(
                    "AllGather", OP.bypass, replica_groups=RG,
                    ins=[ag2_in.opt()], outs=[ag2_out.opt()])
                s1u1f = ap.tile([128, 16], f32, tag="s1u1f")
                nc.sync.dma_start(
                    s1u1f[:, :],
                    ag2_out.rearrange("(r p) j -> p r j", p=128))

                # ------- stage C: [z_pre dz] = W2 @ [s1 u1'] ----------
                pc = pp.tile([16, 2], f32, tag="lat")
                for kc in range(8):
                    nc.tensor.matmul(
                        pc[:, :], lhsT=w2s[:, kc * 16:(kc + 1) * 16],
                        rhs=s1u1f[:, kc * 2:kc * 2 + 2],
                        start=(kc == 0), stop=(kc == 7))
                lat = ap.tile([16, 4], f32, tag="lat_sb")
                nc.vector.tensor_tensor(
                    out=lat[:, 0:1], in0=pc[:, 0:1], in1=b2c[:, :], op=OP.add)
                nc.vector.tensor_copy(lat[:, 1:2], pc[:, 1:2])
                zcol = lat[:, 0:1]

                # ------- SINDy latent: dzb = E_w @ theta(z) + E_b ----------
                p_zr = pp.tile([1, 16], f32, tag="lat")
                nc.tensor.matmul(p_zr[:, :], lhsT=zcol, rhs=i16[:, :],
                                 start=True, stop=True)
                zrow = ap.tile([1, 16], f32, tag="zrow")
                nc.vector.tensor_copy(zrow[:, :], p_zr[:, :])
                p_zz = pp.tile([16, 16], f32, tag="lat")
                nc.tensor.matmul(p_zz[:, :], lhsT=zrow[:, :], rhs=zrow[:, :],
                                 start=True, stop=True)
                zz = ap.tile([16, 16], f32, tag="zz")
                nc.vector.tensor_copy(zz[:, :], p_zz[:, :])
                p_a2 = pp.tile([128, 32], f32, tag="lat")
                nc.tensor.matmul(p_a2[:, 0:16], lhsT=r2a[:, :], rhs=zz[:, :],
                                 start=True, stop=False)
                nc.tensor.matmul(p_a2[:, 16:32], lhsT=r2b[:, :], rhs=zz[:, :],
                                 start=False, stop=True)
                a2 = ap.tile([128, 32], f32, tag="a2")
                nc.vector.tensor_copy(a2[:, :], p_a2[:, :])
                p_zp = pp.tile([128, 1], f32, tag="lat")
                nc.tensor.matmul(p_zp[:, :], lhsT=r1[:, :], rhs=zcol,
                                 start=True, stop=True)
                zrep = ap.tile([128, 1], f32, tag="zrep")
                nc.vector.tensor_copy(zrep[:, :], p_zp[:, :])
                p3v = ap.tile([128, 32], f32, tag="p3v")
                nc.vector.tensor_scalar(
                    out=p3v[:, :], in0=a2[:, :], scalar1=zrep[:, 0:1],
                    scalar2=None, op0=OP.mult)
                p_dzb = pp.tile([16, 1], f32, tag="lat")
                nc.tensor.matmul(p_dzb[:, :], lhsT=ewl[:, :], rhs=zcol,
                                 start=True, stop=False)
                for b in range(16):
                    nc.tensor.matmul(
                        p_dzb[:, :], lhsT=c2s[:, b * 16:(b + 1) * 16],
                        rhs=zz[:, b:b + 1], start=False, stop=False)
                for f in range(32):
                    nc.tensor.matmul(
                        p_dzb[:, :], lhsT=c3s[:, f * 16:(f + 1) * 16],
                        rhs=p3v[:, f:f + 1], start=False, stop=(f == 31))
                nc.vector.tensor_tensor(
                    out=lat[:, 2:3], in0=p_dzb[:, :], in1=c0s[:, :], op=OP.add)

                # ------- stage L0: [t0_pre v0] = wd0 @ [z dzb] ----------
                zdzb = ap.tile([16, 2], f32, tag="zdzb")
                nc.vector.tensor_copy(zdzb[:, 0:1], lat[:, 0:1])
                nc.vector.tensor_copy(zdzb[:, 1:2], lat[:, 2:3])
                pl0 = pp.tile([128, 16], f32, tag="big")
                pl0_r = pview(pl0)
                for mt in range(8):
                    nc.tensor.matmul(
                        pl0[:, mt * 2:(mt + 1) * 2],
                        lhsT=wd0s[:, mt * 128:(mt + 1) * 128], rhs=zdzb[:, :],
                        start=(mt == 0), stop=(mt == 7))
                t0v0 = ap.tile([128, 16], f32, tag="t0v0")
                tmp2 = ap.tile([128, 8], f32, tag="tmp2")
                nc.vector.tensor_tensor(
                    out=tmp2[:, :], in0=pl0_r[:, :, 0], in1=bd0f[:, :],
                    op=OP.add)
                nc.scalar.activation(t0v0[:, 0:8], tmp2[:, :], AF.Sigmoid)
                d2 = ap.tile([128, 8], f32, tag="d2")
                nc.vector.tensor_scalar(
                    out=d2[:, :], in0=t0v0[:, 0:8], scalar1=-1.0, scalar2=1.0,
                    op0=OP.mult, op1=OP.add)
                nc.vector.tensor_tensor(
                    out=d2[:, :], in0=d2[:, :], in1=t0v0[:, 0:8], op=OP.mult)
                nc.vector.tensor_tensor(
                    out=t0v0[:, 8:16], in0=d2[:, :], in1=pl0_r[:, :, 1],
                    op=OP.mult)
                t0v0_r = t0v0.rearrange("p (j c) -> p c j", j=2)

                # ------- stage L1: wd1 shard @ [t0 v0'] (512 rows/core) -----
                pl1 = pp.tile([128, 8], f32, tag="big")
                pl1_r = pview(pl1)
                for kc in range(8):
                    rhs = t0v0_r[:, kc, :]
                    for mt in range(4):
                        nc.tensor.matmul(
                            pl1[:, mt * 2:(mt + 1) * 2],
                            lhsT=wd1s[:, kc * 512 + mt * 128:
                                      kc * 512 + (mt + 1) * 128],
                            rhs=rhs,
                            start=(kc == 0 and mt == 0),
                            stop=(kc == 7 and mt == 3))
                tv = ap.tile([128, 8], f32, tag="tv")
                tmp3 = ap.tile([128, 4], f32, tag="tmp3")
                nc.vector.tensor_tensor(
                    out=tmp3[:, :], in0=pl1_r[:, :, 0], in1=bd1f[:, :],
                    op=OP.add)
                nc.scalar.activation(tv[:, 0:4], tmp3[:, :], AF.Sigmoid)
                d3 = ap.tile([128, 4], f32, tag="d3")
                nc.vector.tensor_scalar(
                    out=d3[:, :], in0=tv[:, 0:4], scalar1=-1.0, scalar2=1.0,
                    op0=OP.mult, op1=OP.add)
                nc.vector.tensor_tensor(
                    out=d3[:, :], in0=d3[:, :], in1=tv[:, 0:4], op=OP.mult)
                nc.vector.tensor_tensor(
                    out=tv[:, 4:8], in0=d3[:, :], in1=pl1_r[:, :, 1],
                    op=OP.mult)

                # ------- AllGather 3: full [t1 v1'] ----------
                ag3_in = dp.tile([128, 8], f32, tag="ag3_in")
                ag3_out = dp.tile([1024, 8], f32, tag="ag3_out")
                nc.sync.dma_start(ag3_in[:, :], tv[:, :])
                nc.gpsimd.collective_compute(
                    "AllGather", OP.bypass, replica_groups=RG,
                    ins=[ag3_in.opt()], outs=[ag3_out.opt()])
                t1full = cp.tile([128, 64], f32, tag="t1full")
                nc.sync.dma_start(
                    t1full[:, :],
                    ag3_out.rearrange("(r p) q -> p r q", p=128))

            # split [t1 v1'] into bf16 hi+lo, laid out per k-chunk:
            # t1hl[p, kc*4 + h*2 + j], kc = r*4 + m
            t1v = t1full.rearrange("p (r j m) -> p r m j", r=8, j=2, m=4)
            t1hl = cp.tile([128, 128], bf16, tag="t1hl")
            t1hl_v = t1hl.rearrange("p (r m h j) -> p h r m j",
                                    r=8, m=4, h=2, j=2)
            nc.vector.tensor_copy(t1hl_v[:, 0], t1v)
            hif32 = ap.tile([128, 64], f32, tag="hif32")
            hif32_v = hif32.rearrange("p (r m j) -> p r m j", r=8, m=4, j=2)
            nc.vector.tensor_copy(hif32_v, t1hl_v[:, 0])
            lof32 = ap.tile([128, 64], f32, tag="lof32")
            lof32_v = lof32.rearrange("p (r m j) -> p r m j", r=8, m=4, j=2)
            nc.vector.tensor_tensor(
                out=lof32_v, in0=t1v, in1=hif32_v, op=OP.subtract)
            nc.vector.tensor_copy(t1hl_v[:, 1], lof32_v)

            with tc.tile_pool(name="ps2", bufs=1, space="PSUM") as pp2:
                # ------- stage E: [xb dxb] = wd2 shard @ [t1 v1'] ----------
                pE = [pp2.tile([2, 512], f32, tag=f"e{nb}", name=f"pE{nb}")
                      for nb in range(4)]
                for kc in range(32):
                    slab = wd2p.tile([128, 4096], bf16, tag="wd2slab")
                    nc.sync.dma_start(
                        slab[:, :], d_wd2[kc].rearrange("h p m -> p h m"))
                    hi = t1hl[:, kc * 4:kc * 4 + 2]
                    lo = t1hl[:, kc * 4 + 2:kc * 4 + 4]
                    for nb in range(4):
                        whi = slab[:, nb * 512:(nb + 1) * 512]
                        wlo = slab[:, 2048 + nb * 512:2048 + (nb + 1) * 512]
                        for i, (a, w) in enumerate(
                                [(hi, whi), (lo, whi), (hi, wlo)]):
                            nc.tensor.matmul(
                                pE[nb][:, :], lhsT=a, rhs=w,
                                start=(kc == 0 and i == 0),
                                stop=(kc == 31 and i == 2))

                for nb in range(4):
                    erows = ap.tile([2, 512], f32, tag="erows", bufs=2,
                                    name=f"erows{nb}")
                    nc.vector.tensor_copy(erows[:, :], pE[nb][:, :])
                    xst = ap.tile([1, 512], f32, tag="xst", bufs=2,
                                  name=f"xst{nb}")
                    nc.vector.tensor_tensor(
                        out=xst[:, :], in0=erows[0:1, :],
                        in1=bd2row[:, nb * 512:(nb + 1) * 512], op=OP.add)
                    nc.sync.dma_start(
                        d_xb.ap()[nb * 512:(nb + 1) * 512], xst[0:1, :])
                    nc.sync.dma_start(
                        d_dxb.ap()[nb * 512:(nb + 1) * 512], erows[1:2, :])
                nc.sync.dma_start(d_lat[:, :], lat[:, 0:3])

    nc.compile()
    return nc


def get_program():
    if "nc" not in _CACHE:
        _CACHE["nc"] = _build_program()
    return _CACHE["nc"]


# ------------------------------------------------------------- host prep

def _split_bf16(a):
    hi = a.astype(BF16)
    lo = (a - hi.astype(np.float32)).astype(BF16)
    return hi, lo


def prepare_in_maps(inputs):
    """Shard + lay out the full inputs into 8 per-core input maps."""
    f = np.float32
    x = np.asarray(inputs["x"], f)
    dx = np.asarray(inputs["dx"], f)
    E_w = np.asarray(inputs["E_w"], f)
    E_b = np.asarray(inputs["E_b"], f)
    we_w0 = np.asarray(inputs["we_w0"], f)
    we_b0 = np.asarray(inputs["we_b0"], f)
    we_w1 = np.asarray(inputs["we_w1"], f)
    we_b1 = np.asarray(inputs["we_b1"], f)
    we_w2 = np.asarray(inputs["we_w2"], f)
    we_b2 = np.asarray(inputs["we_b2"], f)
    wd_w0 = np.asarray(inputs["wd_w0"], f)
    wd_b0 = np.asarray(inputs["wd_b0"], f)
    wd_w1 = np.asarray(inputs["wd_w1"], f)
    wd_b1 = np.asarray(inputs["wd_b1"], f)
    wd_w2 = np.asarray(inputs["wd_w2"], f)
    wd_b2 = np.asarray(inputs["wd_b2"], f)

    # vins[p, kc*4 + h*2 + j]: bf16 hi/lo split of x, dx at index kc*128+p
    xhi, xlo = _split_bf16(x)
    dxhi, dxlo = _split_bf16(dx)
    vins = np.empty((128, 128, 2, 2), BF16)       # [p, kc, h, j]
    vins[:, :, 0, 0] = xhi.reshape(128, 128).T
    vins[:, :, 0, 1] = dxhi.reshape(128, 128).T
    vins[:, :, 1, 0] = xlo.reshape(128, 128).T
    vins[:, :, 1, 1] = dxlo.reshape(128, 128).T
    vins = np.ascontiguousarray(vins.reshape(128, 512))

    # SINDy coefficient folding (replicated)
    c0 = (E_b + E_w[:, 0:L].sum(axis=1)).reshape(16, 1).astype(f)
    ewlint = np.ascontiguousarray(E_w[:, L:2 * L].T)
    c2stack = np.zeros((16, 256), f)
    q = 2 * L
    for i in range(L):
        for j in range(i, L):
            c2stack[i, j * 16:(j + 1) * 16] = E_w[:, q]
            q += 1
    c3stack = np.zeros((128, 512), f)
    for i in range(L):
        for j in range(i, L):
            for k in range(j, L):
                p = i * 8 + j // 2
                fidx = (j % 2) * 16 + k
                c3stack[p, fidx * 16:(fidx + 1) * 16] += E_w[:, q]
                q += 1
    assert q == E_w.shape[1]

    r1 = np.zeros((16, 128), f)
    r1[np.arange(128) // 8, np.arange(128)] = 1.0
    r2a = np.zeros((16, 128), f)
    r2a[2 * (np.arange(128) % 8), np.arange(128)] = 1.0
    r2b = np.zeros((16, 128), f)
    r2b[2 * (np.arange(128) % 8) + 1, np.arange(128)] = 1.0

    shared = {
        "vins": vins,
        "w2t": np.ascontiguousarray(we_w2.T).reshape(8, 128, 16),
        "wd0t": np.ascontiguousarray(wd_w0.T),
        "b2c": we_b2.reshape(16, 1),
        "bd0f": np.ascontiguousarray(wd_b0.reshape(8, 128).T),
        "i16": np.eye(16, dtype=f),
        "r1": r1, "r2a": r2a, "r2b": r2b,
        "c3stack": c3stack, "c2stack": c2stack,
        "ewlint": ewlint, "c0": c0,
    }

    in_maps = []
    for c in range(NCORES):
        r0 = slice(c * H1S, (c + 1) * H1S)        # w0 rows / wd1 rows
        r1b = slice(c * H2S, (c + 1) * H2S)       # w1 rows (128)
        r2 = slice(c * OUTS, (c + 1) * OUTS)      # wd2 rows
        m = dict(shared)

        w0T = np.ascontiguousarray(we_w0[r0, :].T)          # [16384, 512]
        w0hi, w0lo = _split_bf16(w0T)
        w0 = np.empty((128, 2, 128, 512), BF16)             # [kc, h, p, m]
        w0[:, 0] = w0hi.reshape(128, 128, 512)
        w0[:, 1] = w0lo.reshape(128, 128, 512)
        m["w0t"] = np.ascontiguousarray(w0.reshape(32, 4, 2, 128, 512))

        w1T = we_w1[r1b, :].T.reshape(8, 4, 128, 128)      # [r, c, p, m]
        m["w1t"] = np.ascontiguousarray(
            w1T.transpose(1, 0, 2, 3).reshape(32, 128, 128))
        m["wd1t"] = np.ascontiguousarray(wd_w1[r0, :].T).reshape(8, 128, 512)

        wd2T = np.ascontiguousarray(wd_w2[r2, :].T)         # [4096, 2048]
        w2hi, w2lo = _split_bf16(wd2T)
        wd2 = np.empty((32, 2, 128, 2048), BF16)            # [kc, h, p, m]
        wd2[:, 0] = w2hi.reshape(32, 128, 2048)
        wd2[:, 1] = w2lo.reshape(32, 128, 2048)
        m["wd2t"] = np.ascontiguousarray(wd2)

        m["b0row"] = we_b0[r0].reshape(1, 512)
        m["b1sh"] = we_b1[r1b].reshape(128, 1)
        m["bd1f"] = np.ascontiguousarray(wd_b1[r0].reshape(4, 128).T)
        m["bd2row"] = wd_b2[r2].reshape(1, 2048)
        in_maps.append(m)
    return in_maps


def assemble_outputs(results):
    """results: list (per core) of {name: np.ndarray}."""
    lat = np.asarray(results[0]["lat"])
    z = np.ascontiguousarray(lat[:, 0])
    dz = np.ascontiguousarray(lat[:, 1])
    dzb = np.ascontiguousarray(lat[:, 2])
    xb = np.concatenate(
        [np.asarray(results[c]["xb_sh"]).reshape(-1) for c in range(NCORES)])
    dxb = np.concatenate(
        [np.asarray(results[c]["dxb_sh"]).reshape(-1) for c in range(NCORES)])
    return (z, dz, dzb, xb, dxb)


def kernel(**inputs):
    from concourse.bass_utils import run_bass_kernel_spmd
    nc = get_program()
    in_maps = prepare_in_maps(inputs)
    res = run_bass_kernel_spmd(nc, in_maps, core_ids=list(range(NCORES)))
    return assemble_outputs(res.results)


# revision 15
# speedup vs baseline: 1.6249x; 1.0275x over previous
"""SINDy-autoencoder forward+JVP kernel for 8 trn2 NeuronCores.

Network (widths 16384 -> 4096 -> 1024 -> 16, sigmoid; decoder mirrored):
  z   = enc(x)
  dz  = J_enc(x) @ dx          (forward-mode JVP through encoder)
  dzb = E_w @ sindy_library(z) + E_b
  xb  = dec(z)
  dxb = J_dec(z) @ dzb         (JVP through decoder)

Sharding (tensor-parallel, hardcoded for 8 cores):
  we_w0 [4096,16384]  row-sharded 512/core   (stage A, weight-moving)
  we_w1 [1024,4096]   row-sharded 128/core   (stage B; AllGather before+after)
  we_w2 [16,1024]     replicated             (stage C)
  SINDy/latent        replicated
  wd_w0 [1024,16]     replicated             (stage L0)
  wd_w1 [4096,1024]   row-sharded 512/core   (stage L1; AllGather after)
  wd_w2 [16384,4096]  row-sharded 2048/core  (stage E, weight-moving)

The two big stages (A, E) stream the weights as the PE's *moving* operand
(N=512 columns per matmul) against a tiny stationary holding the
[fwd, jvp] activation pair, so the PE consumes 128 weights/cycle instead
of being weight-load bound.  fp32 accuracy is kept by splitting both the
weights and the activations into bf16 hi+lo parts (bf16 upcasts exactly
into the PE's fp22 multiply path) and accumulating the three significant
cross terms (hi*Whi + lo*Whi + hi*Wlo) in fp32 PSUM.

Forward and JVP share every weight byte: each matvec is a matmul with a
2-column stationary pair [fwd, jvp].
"""

import numpy as np
import ml_dtypes

BF16 = ml_dtypes.bfloat16

NCORES = 8
IN, H1, H2, L = 16384, 4096, 1024, 16
H1S = H1 // NCORES      # 512  (encoder w0 rows / decoder wd1 rows per core)
H2S = H2 // NCORES      # 128  (encoder w1 rows per core)
OUTS = IN // NCORES     # 2048 (decoder wd2 rows per core)

WD2_BUFS = 12           # wd2 slab prefetch ring (1MB each)

_CACHE = {}


def _build_program():
    import concourse.bacc as bacc
    import concourse.mybir as mybir
    import concourse.tile as tile

    f32 = mybir.dt.float32
    bf16 = mybir.dt.bfloat16
    AF = mybir.ActivationFunctionType
    OP = mybir.AluOpType

    nc = bacc.Bacc("TRN2", debug=False, target_bir_lowering=False,
                   num_devices=NCORES)

    # ---- I/O ----
    d_vins = nc.dram_tensor("vins", [128, 512], bf16, kind="ExternalInput")
    d_w0 = nc.dram_tensor("w0t", [32, 128, 4096], bf16,
                          kind="ExternalInput")
    d_w1 = nc.dram_tensor("w1t", [128, 4096], f32, kind="ExternalInput")
    d_w2 = nc.dram_tensor("w2t", [128, 128], f32, kind="ExternalInput")
    d_wd0 = nc.dram_tensor("wd0t", [16, 1024], f32, kind="ExternalInput")
    d_wd1 = nc.dram_tensor("wd1t", [128, 4096], f32, kind="ExternalInput")
    d_wd2 = nc.dram_tensor("wd2t", [32, 128, 4096], bf16,
                           kind="ExternalInput")
    d_b0 = nc.dram_tensor("b0row", [1, 512], f32, kind="ExternalInput")
    d_b1 = nc.dram_tensor("b1sh", [128, 1], f32, kind="ExternalInput")
    d_b2 = nc.dram_tensor("b2c", [16, 1], f32, kind="ExternalInput")
    d_bd0 = nc.dram_tensor("bd0f", [128, 8], f32, kind="ExternalInput")
    d_bd1 = nc.dram_tensor("bd1f", [128, 4], f32, kind="ExternalInput")
    d_bd2 = nc.dram_tensor("bd2row", [1, 2048], f32, kind="ExternalInput")
    d_i16 = nc.dram_tensor("i16", [16, 16], f32, kind="ExternalInput")
    d_r1 = nc.dram_tensor("r1", [16, 128], f32, kind="ExternalInput")
    d_r2a = nc.dram_tensor("r2a", [16, 128], f32, kind="ExternalInput")
    d_r2b = nc.dram_tensor("r2b", [16, 128], f32, kind="ExternalInput")
    d_c3 = nc.dram_tensor("c3stack", [128, 512], f32, kind="ExternalInput")
    d_c2 = nc.dram_tensor("c2stack", [16, 256], f32, kind="ExternalInput")
    d_ewl = nc.dram_tensor("ewlint", [16, 16], f32, kind="ExternalInput")
    d_c0 = nc.dram_tensor("c0", [16, 1], f32, kind="ExternalInput")

    d_xb = nc.dram_tensor("xb_sh", [OUTS], f32, kind="ExternalOutput")
    d_dxb = nc.dram_tensor("dxb_sh", [OUTS], f32, kind="ExternalOutput")
    d_lat = nc.dram_tensor("lat", [16, 3], f32, kind="ExternalOutput")

    RG = [list(range(NCORES))]

    with tile.TileContext(nc) as tc:
        with (
            tc.tile_pool(name="consts", bufs=1) as cp,
            tc.tile_pool(name="w0s", bufs=3) as w0p,
            tc.tile_pool(name="wd2s", bufs=WD2_BUFS) as wd2p,
            tc.tile_pool(name="act", bufs=1) as ap,
            tc.tile_pool(name="dram", bufs=1, space="DRAM") as dp,
        ):
            # ---------- constant loads ----------
            vins = cp.tile([128, 512], bf16, tag="vins")
            nc.sync.dma_start(vins[:, :], d_vins[:, :])
            w1s = cp.tile([128, 4096], f32, tag="w1s")
            nc.sync.dma_start(w1s[:, :], d_w1[:, :])
            w2s = cp.tile([128, 128], f32, tag="w2s")
            nc.sync.dma_start(w2s[:, :], d_w2[:, :])
            wd0s = cp.tile([16, 1024], f32, tag="wd0s")
            nc.sync.dma_start(wd0s[:, :], d_wd0[:, :])
            wd1s = cp.tile([128, 4096], f32, tag="wd1s")
            nc.sync.dma_start(wd1s[:, :], d_wd1[:, :])

            def cload(dram, shape, tag, dt=f32):
                t = cp.tile(shape, dt, tag=tag)
                nc.sync.dma_start(t[:, :], dram[:, :])
                return t

            b0row = cload(d_b0, [1, 512], "b0row")
            b1sh = cload(d_b1, [128, 1], "b1sh")
            b2c = cload(d_b2, [16, 1], "b2c")
            bd0f = cload(d_bd0, [128, 8], "bd0f")
            bd1f = cload(d_bd1, [128, 4], "bd1f")
            bd2row = cload(d_bd2, [1, 2048], "bd2row")
            i16 = cload(d_i16, [16, 16], "i16")
            r1 = cload(d_r1, [16, 128], "r1")
            r2a = cload(d_r2a, [16, 128], "r2a")
            r2b = cload(d_r2b, [16, 128], "r2b")
            c3s = cload(d_c3, [128, 512], "c3s")
            c2s = cload(d_c2, [16, 256], "c2s")
            ewl = cload(d_ewl, [16, 16], "ewl")
            c0s = cload(d_c0, [16, 1], "c0s")

            def pview(t):
                # psum pair-major [P, (m j)] -> [P, m, j]
                return t.rearrange("p (m j) -> p m j", j=2)

            # Tiny dummy collective issued up front: absorbs the ncfw
            # first-use cost (~40us) while stage A streams weights.
            warm_in = dp.tile([1, 8], f32, tag="warm_in")
            warm_out = dp.tile([8, 8], f32, tag="warm_out")
            warm_sb = ap.tile([1, 8], f32, tag="warm_sb")
            nc.vector.memset(warm_sb[:, :], 0.0)
            nc.sync.dma_start(warm_in[:, :], warm_sb[:, :])
            nc.gpsimd.collective_compute(
                "AllGather", OP.bypass, replica_groups=RG,
                ins=[warm_in.opt()], outs=[warm_out.opt()])

            with tc.tile_pool(name="ps1", bufs=1, space="PSUM") as pp:
                # ------- stage A: [a0; u0] = W0s @ [x dx] (weight-moving) ----
                pA = pp.tile([2, 512], f32, tag="row")
                for g in range(32):          # 32 slabs x (4 kc x 2 h) x 1MB
                    slab = w0p.tile([128, 4096], bf16, tag="w0slab")
                    nc.sync.dma_start(slab[:, :], d_w0[g])
                    for ks in range(4):
                        kc = g * 4 + ks
                        hi = vins[:, kc * 4:kc * 4 + 2]
                        lo = vins[:, kc * 4 + 2:kc * 4 + 4]
                        whi = slab[:, ks * 1024:ks * 1024 + 512]
                        wlo = slab[:, ks * 1024 + 512:ks * 1024 + 1024]
                        for i, (a, w) in enumerate(
                                [(hi, whi), (lo, whi), (hi, wlo)]):
                            nc.tensor.matmul(
                                pA[:, :], lhsT=a, rhs=w,
                                start=(kc == 0 and i == 0),
                                stop=(kc == 127 and i == 2))

                # s0 = sigmoid(a0 + b0), u0' = s0(1-s0)*u0   (row layout)
                rows2 = ap.tile([2, 512], f32, tag="rows2")
                nc.vector.tensor_copy(rows2[:, :], pA[:, :])
                u0raw = ap.tile([1, 512], f32, tag="u0raw")
                nc.sync.dma_start(u0raw[:, :], rows2[1:2, :])
                s0row = ap.tile([1, 512], f32, tag="s0row")
                tmpr = ap.tile([1, 512], f32, tag="tmpr")
                nc.vector.tensor_tensor(
                    out=tmpr[:, :], in0=rows2[0:1, :], in1=b0row[:, :],
                    op=OP.add)
                nc.scalar.activation(s0row[:, :], tmpr[:, :], AF.Sigmoid)
                d0r = ap.tile([1, 512], f32, tag="d0r")
                nc.vector.tensor_scalar(
                    out=d0r[:, :], in0=s0row[:, :], scalar1=-1.0,
                    scalar2=1.0, op0=OP.mult, op1=OP.add)
                nc.vector.tensor_tensor(
                    out=d0r[:, :], in0=d0r[:, :], in1=s0row[:, :],
                    op=OP.mult)
                u0p = ap.tile([1, 512], f32, tag="u0p")
                nc.vector.tensor_tensor(
                    out=u0p[:, :], in0=d0r[:, :], in1=u0raw[:, :],
                    op=OP.mult)

                # ------- AllGather 1: full [s0; u0'] ----------
                ag1_in = dp.tile([2, 512], f32, tag="ag1_in")
                ag1_out = dp.tile([16, 512], f32, tag="ag1_out")
                nc.sync.dma_start(ag1_in[0:1, :], s0row[:, :])
                nc.sync.dma_start(ag1_in[1:2, :], u0p[:, :])
                nc.gpsimd.collective_compute(
                    "AllGather", OP.bypass, replica_groups=RG,
                    ins=[ag1_in.opt()], outs=[ag1_out.opt()])
                agsb = ap.tile([16, 512], f32, tag="agsb")
                nc.sync.dma_start(agsb[:, :], ag1_out[:, :])
                # transpose to column layout: s0u0f[p, c*16 + r*2 + j]
                # (stage-B chunk kc = c*8 + r covers s0 indices r*512+c*128+p)
                s0u0f = ap.tile([128, 64], f32, tag="s0u0f")
                for c in range(4):
                    ptr = pp.tile([128, 16], f32, tag="lat", name=f"ptr{c}")
                    nc.tensor.transpose(
                        ptr[:, :], agsb[:, c * 128:(c + 1) * 128], i16[:, :])
                    nc.vector.tensor_copy(
                        s0u0f[:, c * 16:(c + 1) * 16], ptr[:, :])

                # ------- stage B: a1 shard = W1s(128 rows) @ [s0 u0'] -------
                pB = pp.tile([128, 2], f32, tag="big")
                for kc in range(32):
                    nc.tensor.matmul(
                        pB[:, :], lhsT=w1s[:, kc * 128:(kc + 1) * 128],
                        rhs=s0u0f[:, kc * 2:kc * 2 + 2],
                        start=(kc == 0), stop=(kc == 31))
                s1u1sh = ap.tile([128, 2], f32, tag="s1u1sh")
                tmps = ap.tile([128, 1], f32, tag="tmps")
                nc.vector.tensor_tensor(
                    out=tmps[:, :], in0=pB[:, 0:1], in1=b1sh[:, :], op=OP.add)
                nc.scalar.activation(s1u1sh[:, 0:1], tmps[:, :], AF.Sigmoid)
                d1s = ap.tile([128, 1], f32, tag="d1s")
                nc.vector.tensor_scalar(
                    out=d1s[:, :], in0=s1u1sh[:, 0:1], scalar1=-1.0,
                    scalar2=1.0, op0=OP.mult, op1=OP.add)
                nc.vector.tensor_tensor(
                    out=d1s[:, :], in0=d1s[:, :], in1=s1u1sh[:, 0:1],
                    op=OP.mult)
                nc.vector.tensor_tensor(
                    out=s1u1sh[:, 1:2], in0=d1s[:, :], in1=pB[:, 1:2],
                    op=OP.mult)

                # ------- AllGather 2: full [s1; u1'] ----------
                ag2_in = dp.tile([128, 2], f32, tag="ag2_in")
                ag2_out = dp.tile([1024, 2], f32, tag="ag2_out")
                nc.sync.dma_start(ag2_in[:, :], s1u1sh[:, :])
                nc.gpsimd.collective_compute(
                    "AllGather", OP.bypass, replica_groups=RG,
                    ins=[ag2_in.opt()], outs=[ag2_out.opt()])
                s1u1f = ap.tile([128, 16], f32, tag="s1u1f")
                nc.sync.dma_start(
                    s1u1f[:, :],
                    ag2_out.rearrange("(r p) j -> p r j", p=128))

                # ------- stage C: [z_pre dz] = W2 @ [s1 u1'] ----------
                pc = pp.tile([16, 2], f32, tag="lat")
                for kc in range(8):
                    nc.tensor.matmul(
                        pc[:, :], lhsT=w2s[:, kc * 16:(kc + 1) * 16],
                        rhs=s1u1f[:, kc * 2:kc * 2 + 2],
                        start=(kc == 0), stop=(kc == 7))
                lat = ap.tile([16, 4], f32, tag="lat_sb")
                nc.vector.tensor_tensor(
                    out=lat[:, 0:1], in0=pc[:, 0:1], in1=b2c[:, :], op=OP.add)
                nc.vector.tensor_copy(lat[:, 1:2], pc[:, 1:2])
                zcol = lat[:, 0:1]

                # ------- SINDy latent: dzb = E_w @ theta(z) + E_b ----------
                p_zr = pp.tile([1, 16], f32, tag="lat")
                nc.tensor.matmul(p_zr[:, :], lhsT=zcol, rhs=i16[:, :],
                                 start=True, stop=True)
                zrow = ap.tile([1, 16], f32, tag="zrow")
                nc.vector.tensor_copy(zrow[:, :], p_zr[:, :])
                p_zz = pp.tile([16, 16], f32, tag="lat")
                nc.tensor.matmul(p_zz[:, :], lhsT=zrow[:, :], rhs=zrow[:, :],
                                 start=True, stop=True)
                zz = ap.tile([16, 16], f32, tag="zz")
                nc.vector.tensor_copy(zz[:, :], p_zz[:, :])
                p_a2 = pp.tile([128, 32], f32, tag="lat")
                nc.tensor.matmul(p_a2[:, 0:16], lhsT=r2a[:, :], rhs=zz[:, :],
                                 start=True, stop=False)
                nc.tensor.matmul(p_a2[:, 16:32], lhsT=r2b[:, :], rhs=zz[:, :],
                                 start=False, stop=True)
                a2 = ap.tile([128, 32], f32, tag="a2")
                nc.vector.tensor_copy(a2[:, :], p_a2[:, :])
                p_zp = pp.tile([128, 1], f32, tag="lat")
                nc.tensor.matmul(p_zp[:, :], lhsT=r1[:, :], rhs=zcol,
                                 start=True, stop=True)
                zrep = ap.tile([128, 1], f32, tag="zrep")
                nc.vector.tensor_copy(zrep[:, :], p_zp[:, :])
                p3v = ap.tile([128, 32], f32, tag="p3v")
                nc.vector.tensor_scalar(
                    out=p3v[:, :], in0=a2[:, :], scalar1=zrep[:, 0:1],
                    scalar2=None, op0=OP.mult)
                p_dzb = pp.tile([16, 1], f32, tag="lat")
                nc.tensor.matmul(p_dzb[:, :], lhsT=ewl[:, :], rhs=zcol,
                                 start=True, stop=False)
                for b in range(16):
                    nc.tensor.matmul(
                        p_dzb[:, :], lhsT=c2s[:, b * 16:(b + 1) * 16],
                        rhs=zz[:, b:b + 1], start=False, stop=False)
                for f in range(32):
                    nc.tensor.matmul(
                        p_dzb[:, :], lhsT=c3s[:, f * 16:(f + 1) * 16],
                        rhs=p3v[:, f:f + 1], start=False, stop=(f == 31))
                nc.vector.tensor_tensor(
                    out=lat[:, 2:3], in0=p_dzb[:, :], in1=c0s[:, :], op=OP.add)

                # ------- stage L0: [t0_pre v0] = wd0 @ [z dzb] ----------
                zdzb = ap.tile([16, 2], f32, tag="zdzb")
                nc.vector.tensor_copy(zdzb[:, 0:1], lat[:, 0:1])
                nc.vector.tensor_copy(zdzb[:, 1:2], lat[:, 2:3])
                pl0 = pp.tile([128, 16], f32, tag="big")
                pl0_r = pview(pl0)
                for mt in range(8):
                    nc.tensor.matmul(
                        pl0[:, mt * 2:(mt + 1) * 2],
                        lhsT=wd0s[:, mt * 128:(mt + 1) * 128], rhs=zdzb[:, :],
                        start=(mt == 0), stop=(mt == 7))
                t0v0 = ap.tile([128, 16], f32, tag="t0v0")
                tmp2 = ap.tile([128, 8], f32, tag="tmp2")
                nc.vector.tensor_tensor(
                    out=tmp2[:, :], in0=pl0_r[:, :, 0], in1=bd0f[:, :],
                    op=OP.add)
                nc.scalar.activation(t0v0[:, 0:8], tmp2[:, :], AF.Sigmoid)
                d2 = ap.tile([128, 8], f32, tag="d2")
                nc.vector.tensor_scalar(
                    out=d2[:, :], in0=t0v0[:, 0:8], scalar1=-1.0, scalar2=1.0,
                    op0=OP.mult, op1=OP.add)
                nc.vector.tensor_tensor(
                    out=d2[:, :], in0=d2[:, :], in1=t0v0[:, 0:8], op=OP.mult)
                nc.vector.tensor_tensor(
                    out=t0v0[:, 8:16], in0=d2[:, :], in1=pl0_r[:, :, 1],
                    op=OP.mult)
                t0v0_r = t0v0.rearrange("p (j c) -> p c j", j=2)

                # ------- stage L1: wd1 shard @ [t0 v0'] (512 rows/core) -----
                pl1 = pp.tile([128, 8], f32, tag="big")
                pl1_r = pview(pl1)
                for kc in range(8):
                    rhs = t0v0_r[:, kc, :]
                    for mt in range(4):
                        nc.tensor.matmul(
                            pl1[:, mt * 2:(mt + 1) * 2],
                            lhsT=wd1s[:, kc * 512 + mt * 128:
                                      kc * 512 + (mt + 1) * 128],
                            rhs=rhs,
                            start=(kc == 0 and mt == 0),
                            stop=(kc == 7 and mt == 3))
                tv = ap.tile([128, 8], f32, tag="tv")
                tmp3 = ap.tile([128, 4], f32, tag="tmp3")
                nc.vector.tensor_tensor(
                    out=tmp3[:, :], in0=pl1_r[:, :, 0], in1=bd1f[:, :],
                    op=OP.add)
                nc.scalar.activation(tv[:, 0:4], tmp3[:, :], AF.Sigmoid)
                d3 = ap.tile([128, 4], f32, tag="d3")
                nc.vector.tensor_scalar(
                    out=d3[:, :], in0=tv[:, 0:4], scalar1=-1.0, scalar2=1.0,
                    op0=OP.mult, op1=OP.add)
                nc.vector.tensor_tensor(
                    out=d3[:, :], in0=d3[:, :], in1=tv[:, 0:4], op=OP.mult)
                nc.vector.tensor_tensor(
                    out=tv[:, 4:8], in0=d3[:, :], in1=pl1_r[:, :, 1],
                    op=OP.mult)

                # ------- AllGather 3: full [t1 v1'] ----------
                ag3_in = dp.tile([128, 8], f32, tag="ag3_in")
                ag3_out = dp.tile([1024, 8], f32, tag="ag3_out")
                nc.sync.dma_start(ag3_in[:, :], tv[:, :])
                nc.gpsimd.collective_compute(
                    "AllGather", OP.bypass, replica_groups=RG,
                    ins=[ag3_in.opt()], outs=[ag3_out.opt()])
                t1full = cp.tile([128, 64], f32, tag="t1full")
                nc.sync.dma_start(
                    t1full[:, :],
                    ag3_out.rearrange("(r p) q -> p r q", p=128))

            # split [t1 v1'] into bf16 hi+lo, laid out per k-chunk:
            # t1hl[p, kc*4 + h*2 + j], kc = r*4 + m
            t1v = t1full.rearrange("p (r j m) -> p r m j", r=8, j=2, m=4)
            t1hl = cp.tile([128, 128], bf16, tag="t1hl")
            t1hl_v = t1hl.rearrange("p (r m h j) -> p h r m j",
                                    r=8, m=4, h=2, j=2)
            nc.vector.tensor_copy(t1hl_v[:, 0], t1v)
            hif32 = ap.tile([128, 64], f32, tag="hif32")
            hif32_v = hif32.rearrange("p (r m j) -> p r m j", r=8, m=4, j=2)
            nc.vector.tensor_copy(hif32_v, t1hl_v[:, 0])
            lof32 = ap.tile([128, 64], f32, tag="lof32")
            lof32_v = lof32.rearrange("p (r m j) -> p r m j", r=8, m=4, j=2)
            nc.vector.tensor_tensor(
                out=lof32_v, in0=t1v, in1=hif32_v, op=OP.subtract)
            nc.vector.tensor_copy(t1hl_v[:, 1], lof32_v)

            with tc.tile_pool(name="ps2", bufs=1, space="PSUM") as pp2:
                # ------- stage E: [xb dxb] = wd2 shard @ [t1 v1'] ----------
                pE = [pp2.tile([2, 512], f32, tag=f"e{nb}", name=f"pE{nb}")
                      for nb in range(4)]
                for kc in range(32):
                    slab = wd2p.tile([128, 4096], bf16, tag="wd2slab")
                    nc.sync.dma_start(slab[:, :], d_wd2[kc])
                    hi = t1hl[:, kc * 4:kc * 4 + 2]
                    lo = t1hl[:, kc * 4 + 2:kc * 4 + 4]
                    for nb in range(4):
                        whi = slab[:, nb * 512:(nb + 1) * 512]
                        wlo = slab[:, 2048 + nb * 512:2048 + (nb + 1) * 512]
                        for i, (a, w) in enumerate(
                                [(hi, whi), (lo, whi), (hi, wlo)]):
                            nc.tensor.matmul(
                                pE[nb][:, :], lhsT=a, rhs=w,
                                start=(kc == 0 and i == 0),
                                stop=(kc == 31 and i == 2))

                for nb in range(4):
                    erows = ap.tile([2, 512], f32, tag="erows", bufs=2,
                                    name=f"erows{nb}")
                    nc.vector.tensor_copy(erows[:, :], pE[nb][:, :])
                    xst = ap.tile([1, 512], f32, tag="xst", bufs=2,
                                  name=f"xst{nb}")
                    nc.vector.tensor_tensor(
                        out=xst[:, :], in0=erows[0:1, :],
                        in1=bd2row[:, nb * 512:(nb + 1) * 512], op=OP.add)
                    nc.sync.dma_start(
                        d_xb.ap()[nb * 512:(nb + 1) * 512], xst[0:1, :])
                    nc.sync.dma_start(
                        d_dxb.ap()[nb * 512:(nb + 1) * 512], erows[1:2, :])
                nc.sync.dma_start(d_lat[:, :], lat[:, 0:3])

    nc.compile()
    return nc


def get_program():
    if "nc" not in _CACHE:
        _CACHE["nc"] = _build_program()
    return _CACHE["nc"]


# ------------------------------------------------------------- host prep

def _split_bf16(a):
    hi = a.astype(BF16)
    lo = (a - hi.astype(np.float32)).astype(BF16)
    return hi, lo


def prepare_in_maps(inputs):
    """Shard + lay out the full inputs into 8 per-core input maps."""
    f = np.float32
    x = np.asarray(inputs["x"], f)
    dx = np.asarray(inputs["dx"], f)
    E_w = np.asarray(inputs["E_w"], f)
    E_b = np.asarray(inputs["E_b"], f)
    we_w0 = np.asarray(inputs["we_w0"], f)
    we_b0 = np.asarray(inputs["we_b0"], f)
    we_w1 = np.asarray(inputs["we_w1"], f)
    we_b1 = np.asarray(inputs["we_b1"], f)
    we_w2 = np.asarray(inputs["we_w2"], f)
    we_b2 = np.asarray(inputs["we_b2"], f)
    wd_w0 = np.asarray(inputs["wd_w0"], f)
    wd_b0 = np.asarray(inputs["wd_b0"], f)
    wd_w1 = np.asarray(inputs["wd_w1"], f)
    wd_b1 = np.asarray(inputs["wd_b1"], f)
    wd_w2 = np.asarray(inputs["wd_w2"], f)
    wd_b2 = np.asarray(inputs["wd_b2"], f)

    # vins[p, kc*4 + h*2 + j]: bf16 hi/lo split of x, dx at index kc*128+p
    xhi, xlo = _split_bf16(x)
    dxhi, dxlo = _split_bf16(dx)
    vins = np.empty((128, 128, 2, 2), BF16)       # [p, kc, h, j]
    vins[:, :, 0, 0] = xhi.reshape(128, 128).T
    vins[:, :, 0, 1] = dxhi.reshape(128, 128).T
    vins[:, :, 1, 0] = xlo.reshape(128, 128).T
    vins[:, :, 1, 1] = dxlo.reshape(128, 128).T
    vins = np.ascontiguousarray(vins.reshape(128, 512))

    # SINDy coefficient folding (replicated)
    c0 = (E_b + E_w[:, 0:L].sum(axis=1)).reshape(16, 1).astype(f)
    ewlint = np.ascontiguousarray(E_w[:, L:2 * L].T)
    c2stack = np.zeros((16, 256), f)
    q = 2 * L
    for i in range(L):
        for j in range(i, L):
            c2stack[i, j * 16:(j + 1) * 16] = E_w[:, q]
            q += 1
    c3stack = np.zeros((128, 512), f)
    for i in range(L):
        for j in range(i, L):
            for k in range(j, L):
                p = i * 8 + j // 2
                fidx = (j % 2) * 16 + k
                c3stack[p, fidx * 16:(fidx + 1) * 16] += E_w[:, q]
                q += 1
    assert q == E_w.shape[1]

    r1 = np.zeros((16, 128), f)
    r1[np.arange(128) // 8, np.arange(128)] = 1.0
    r2a = np.zeros((16, 128), f)
    r2a[2 * (np.arange(128) % 8), np.arange(128)] = 1.0
    r2b = np.zeros((16, 128), f)
    r2b[2 * (np.arange(128) % 8) + 1, np.arange(128)] = 1.0

    shared = {
        "vins": vins,
        "w2t": np.ascontiguousarray(
            we_w2.T.reshape(8, 128, 16).transpose(1, 0, 2).reshape(128, 128)),
        "wd0t": np.ascontiguousarray(wd_w0.T),
        "b2c": we_b2.reshape(16, 1),
        "bd0f": np.ascontiguousarray(wd_b0.reshape(8, 128).T),
        "i16": np.eye(16, dtype=f),
        "r1": r1, "r2a": r2a, "r2b": r2b,
        "c3stack": c3stack, "c2stack": c2stack,
        "ewlint": ewlint, "c0": c0,
    }

    in_maps = []
    for c in range(NCORES):
        r0 = slice(c * H1S, (c + 1) * H1S)        # w0 rows / wd1 rows
        r1b = slice(c * H2S, (c + 1) * H2S)       # w1 rows (128)
        r2 = slice(c * OUTS, (c + 1) * OUTS)      # wd2 rows
        m = dict(shared)

        w0T = np.ascontiguousarray(we_w0[r0, :].T)          # [16384, 512]
        w0hi, w0lo = _split_bf16(w0T)
        # slab layout [g, p, (k, h, m)] so each partition row is contiguous
        w0 = np.empty((32, 4, 2, 128, 512), BF16)           # [g, k, h, p, m]
        w0[:, :, 0] = w0hi.reshape(32, 4, 128, 512)
        w0[:, :, 1] = w0lo.reshape(32, 4, 128, 512)
        m["w0t"] = np.ascontiguousarray(
            w0.transpose(0, 3, 1, 2, 4).reshape(32, 128, 4096))

        w1T = we_w1[r1b, :].T.reshape(8, 4, 128, 128)      # [r, c, p, m]
        m["w1t"] = np.ascontiguousarray(
            w1T.transpose(2, 1, 0, 3).reshape(128, 4096))
        m["wd1t"] = np.ascontiguousarray(
            wd_w1[r0, :].T.reshape(8, 128, 512).transpose(1, 0, 2).reshape(
                128, 4096))

        wd2T = np.ascontiguousarray(wd_w2[r2, :].T)         # [4096, 2048]
        w2hi, w2lo = _split_bf16(wd2T)
        wd2 = np.empty((32, 2, 128, 2048), BF16)            # [kc, h, p, m]
        wd2[:, 0] = w2hi.reshape(32, 128, 2048)
        wd2[:, 1] = w2lo.reshape(32, 128, 2048)
        m["wd2t"] = np.ascontiguousarray(
            wd2.transpose(0, 2, 1, 3).reshape(32, 128, 4096))

        m["b0row"] = we_b0[r0].reshape(1, 512)
        m["b1sh"] = we_b1[r1b].reshape(128, 1)
        m["bd1f"] = np.ascontiguousarray(wd_b1[r0].reshape(4, 128).T)
        m["bd2row"] = wd_b2[r2].reshape(1, 2048)
        in_maps.append(m)
    return in_maps


def assemble_outputs(results):
    """results: list (per core) of {name: np.ndarray}."""
    lat = np.asarray(results[0]["lat"])
    z = np.ascontiguousarray(lat[:, 0])
    dz = np.ascontiguousarray(lat[:, 1])
    dzb = np.ascontiguousarray(lat[:, 2])
    xb = np.concatenate(
        [np.asarray(results[c]["xb_sh"]).reshape(-1) for c in range(NCORES)])
    dxb = np.concatenate(
        [np.asarray(results[c]["dxb_sh"]).reshape(-1) for c in range(NCORES)])
    return (z, dz, dzb, xb, dxb)


def kernel(**inputs):
    from concourse.bass_utils import run_bass_kernel_spmd
    nc = get_program()
    in_maps = prepare_in_maps(inputs)
    res = run_bass_kernel_spmd(nc, in_maps, core_ids=list(range(NCORES)))
    return assemble_outputs(res.results)


# revision 16
# speedup vs baseline: 1.7127x; 1.0540x over previous
"""SINDy-autoencoder forward+JVP kernel for 8 trn2 NeuronCores.

Network (widths 16384 -> 4096 -> 1024 -> 16, sigmoid; decoder mirrored):
  z   = enc(x)
  dz  = J_enc(x) @ dx          (forward-mode JVP through encoder)
  dzb = E_w @ sindy_library(z) + E_b
  xb  = dec(z)
  dxb = J_dec(z) @ dzb         (JVP through decoder)

Sharding (tensor-parallel, hardcoded for 8 cores):
  we_w0 [4096,16384]  row-sharded 512/core   (stage A, weight-moving)
  we_w1 [1024,4096]   row-sharded 128/core   (stage B; AllGather before+after)
  we_w2 [16,1024]     replicated             (stage C)
  SINDy/latent        replicated
  wd_w0 [1024,16]     replicated             (stage L0)
  wd_w1 [4096,1024]   row-sharded 512/core   (stage L1; AllGather after)
  wd_w2 [16384,4096]  row-sharded 2048/core  (stage E, weight-moving)

The two big stages (A, E) stream the weights as the PE's *moving* operand
(N=512 columns per matmul) against a tiny stationary holding the
[fwd, jvp] activation pair, so the PE consumes 128 weights/cycle instead
of being weight-load bound.  fp32 accuracy is kept by splitting both the
weights and the activations into bf16 hi+lo parts (bf16 upcasts exactly
into the PE's fp22 multiply path) and accumulating the three significant
cross terms (hi*Whi + lo*Whi + hi*Wlo) in fp32 PSUM.

Forward and JVP share every weight byte: each matvec is a matmul with a
2-column stationary pair [fwd, jvp].
"""

import numpy as np
import ml_dtypes

BF16 = ml_dtypes.bfloat16

NCORES = 8
IN, H1, H2, L = 16384, 4096, 1024, 16
H1S = H1 // NCORES      # 512  (encoder w0 rows / decoder wd1 rows per core)
H2S = H2 // NCORES      # 128  (encoder w1 rows per core)
OUTS = IN // NCORES     # 2048 (decoder wd2 rows per core)

WD2_BUFS = 12           # wd2 slab prefetch ring (1MB each)

_CACHE = {}


def _build_program():
    import concourse.bacc as bacc
    import concourse.mybir as mybir
    import concourse.tile as tile

    f32 = mybir.dt.float32
    bf16 = mybir.dt.bfloat16
    AF = mybir.ActivationFunctionType
    OP = mybir.AluOpType

    nc = bacc.Bacc("TRN2", debug=False, target_bir_lowering=False,
                   num_devices=NCORES)

    # ---- I/O ----
    d_vins = nc.dram_tensor("vins", [128, 512], bf16, kind="ExternalInput")
    d_w0 = nc.dram_tensor("w0t", [32, 128, 4096], bf16,
                          kind="ExternalInput")
    d_w1 = nc.dram_tensor("w1t", [128, 4096], f32, kind="ExternalInput")
    d_w2 = nc.dram_tensor("w2t", [128, 128], f32, kind="ExternalInput")
    d_wd0 = nc.dram_tensor("wd0t", [16, 1024], f32, kind="ExternalInput")
    d_wd1 = nc.dram_tensor("wd1t", [128, 4096], f32, kind="ExternalInput")
    d_wd2 = nc.dram_tensor("wd2t", [32, 128, 4096], bf16,
                           kind="ExternalInput")
    d_b0 = nc.dram_tensor("b0row", [1, 512], f32, kind="ExternalInput")
    d_b1 = nc.dram_tensor("b1sh", [128, 1], f32, kind="ExternalInput")
    d_b2 = nc.dram_tensor("b2c", [16, 1], f32, kind="ExternalInput")
    d_bd0 = nc.dram_tensor("bd0f", [128, 8], f32, kind="ExternalInput")
    d_bd1 = nc.dram_tensor("bd1f", [128, 4], f32, kind="ExternalInput")
    d_bd2 = nc.dram_tensor("bd2row", [1, 2048], f32, kind="ExternalInput")
    d_i16 = nc.dram_tensor("i16", [16, 16], f32, kind="ExternalInput")
    d_r1 = nc.dram_tensor("r1", [16, 128], f32, kind="ExternalInput")
    d_r2a = nc.dram_tensor("r2a", [16, 128], f32, kind="ExternalInput")
    d_r2b = nc.dram_tensor("r2b", [16, 128], f32, kind="ExternalInput")
    d_c3 = nc.dram_tensor("c3stack", [128, 512], f32, kind="ExternalInput")
    d_c2 = nc.dram_tensor("c2stack", [16, 256], f32, kind="ExternalInput")
    d_ewl = nc.dram_tensor("ewlint", [16, 16], f32, kind="ExternalInput")
    d_c0 = nc.dram_tensor("c0", [16, 1], f32, kind="ExternalInput")
    d_c4 = nc.dram_tensor("c4comb", [4, 2], f32, kind="ExternalInput")

    d_xb = nc.dram_tensor("xb_sh", [OUTS], f32, kind="ExternalOutput")
    d_dxb = nc.dram_tensor("dxb_sh", [OUTS], f32, kind="ExternalOutput")
    d_lat = nc.dram_tensor("lat", [16, 3], f32, kind="ExternalOutput")

    RG = [list(range(NCORES))]

    with tile.TileContext(nc) as tc:
        with (
            tc.tile_pool(name="consts", bufs=1) as cp,
            tc.tile_pool(name="w0s", bufs=3) as w0p,
            tc.tile_pool(name="wd2s", bufs=WD2_BUFS) as wd2p,
            tc.tile_pool(name="act", bufs=1) as ap,
            tc.tile_pool(name="dram", bufs=1, space="DRAM") as dp,
        ):
            # ---------- constant loads ----------
            vins = cp.tile([128, 512], bf16, tag="vins")
            nc.sync.dma_start(vins[:, :], d_vins[:, :])
            w1s = cp.tile([128, 4096], f32, tag="w1s")
            nc.sync.dma_start(w1s[:, :], d_w1[:, :])
            w2s = cp.tile([128, 128], f32, tag="w2s")
            nc.sync.dma_start(w2s[:, :], d_w2[:, :])
            wd0s = cp.tile([16, 1024], f32, tag="wd0s")
            nc.sync.dma_start(wd0s[:, :], d_wd0[:, :])
            wd1s = cp.tile([128, 4096], f32, tag="wd1s")
            nc.sync.dma_start(wd1s[:, :], d_wd1[:, :])

            def cload(dram, shape, tag, dt=f32):
                t = cp.tile(shape, dt, tag=tag)
                nc.sync.dma_start(t[:, :], dram[:, :])
                return t

            b0row = cload(d_b0, [1, 512], "b0row")
            b1sh = cload(d_b1, [128, 1], "b1sh")
            b2c = cload(d_b2, [16, 1], "b2c")
            bd0f = cload(d_bd0, [128, 8], "bd0f")
            bd1f = cload(d_bd1, [128, 4], "bd1f")
            bd2row = cload(d_bd2, [1, 2048], "bd2row")
            i16 = cload(d_i16, [16, 16], "i16")
            r1 = cload(d_r1, [16, 128], "r1")
            r2a = cload(d_r2a, [16, 128], "r2a")
            r2b = cload(d_r2b, [16, 128], "r2b")
            c3s = cload(d_c3, [128, 512], "c3s")
            c2s = cload(d_c2, [16, 256], "c2s")
            ewl = cload(d_ewl, [16, 16], "ewl")
            c0s = cload(d_c0, [16, 1], "c0s")
            c4c = cload(d_c4, [4, 2], "c4comb")

            def pview(t):
                # psum pair-major [P, (m j)] -> [P, m, j]
                return t.rearrange("p (m j) -> p m j", j=2)

            # Tiny dummy collective issued up front: absorbs the ncfw
            # first-use cost (~40us) while stage A streams weights.
            warm_in = dp.tile([1, 8], f32, tag="warm_in")
            warm_out = dp.tile([8, 8], f32, tag="warm_out")
            warm_sb = ap.tile([1, 8], f32, tag="warm_sb")
            nc.vector.memset(warm_sb[:, :], 0.0)
            nc.sync.dma_start(warm_in[:, :], warm_sb[:, :])
            nc.gpsimd.collective_compute(
                "AllGather", OP.bypass, replica_groups=RG,
                ins=[warm_in.opt()], outs=[warm_out.opt()])

            with tc.tile_pool(name="ps1", bufs=1, space="PSUM") as pp:
                # ------- stage A: [a0; u0] = W0s @ [x dx] (weight-moving) ----
                pA = pp.tile([2, 512], f32, tag="row")
                for g in range(32):          # 32 slabs x (4 kc x 2 h) x 1MB
                    slab = w0p.tile([128, 4096], bf16, tag="w0slab")
                    nc.sync.dma_start(slab[:, :], d_w0[g])
                    for ks in range(4):
                        kc = g * 4 + ks
                        hi = vins[:, kc * 4:kc * 4 + 2]
                        lo = vins[:, kc * 4 + 2:kc * 4 + 4]
                        whi = slab[:, ks * 1024:ks * 1024 + 512]
                        wlo = slab[:, ks * 1024 + 512:ks * 1024 + 1024]
                        for i, (a, w) in enumerate(
                                [(hi, whi), (lo, whi), (hi, wlo)]):
                            nc.tensor.matmul(
                                pA[:, :], lhsT=a, rhs=w,
                                start=(kc == 0 and i == 0),
                                stop=(kc == 127 and i == 2))

                # s0 = sigmoid(a0 + b0), u0' = s0(1-s0)*u0   (row layout)
                rows2 = ap.tile([2, 512], f32, tag="rows2")
                nc.vector.tensor_copy(rows2[:, :], pA[:, :])
                u0raw = ap.tile([1, 512], f32, tag="u0raw")
                nc.sync.dma_start(u0raw[:, :], rows2[1:2, :])
                s0row = ap.tile([1, 512], f32, tag="s0row")
                tmpr = ap.tile([1, 512], f32, tag="tmpr")
                nc.vector.tensor_tensor(
                    out=tmpr[:, :], in0=rows2[0:1, :], in1=b0row[:, :],
                    op=OP.add)
                nc.scalar.activation(s0row[:, :], tmpr[:, :], AF.Sigmoid)
                d0r = ap.tile([1, 512], f32, tag="d0r")
                nc.vector.tensor_scalar(
                    out=d0r[:, :], in0=s0row[:, :], scalar1=-1.0,
                    scalar2=1.0, op0=OP.mult, op1=OP.add)
                nc.vector.tensor_tensor(
                    out=d0r[:, :], in0=d0r[:, :], in1=s0row[:, :],
                    op=OP.mult)
                u0p = ap.tile([1, 512], f32, tag="u0p")
                nc.vector.tensor_tensor(
                    out=u0p[:, :], in0=d0r[:, :], in1=u0raw[:, :],
                    op=OP.mult)

                # ------- AllGather 1: full [s0; u0'] ----------
                ag1_in = dp.tile([2, 512], f32, tag="ag1_in")
                ag1_out = dp.tile([16, 512], f32, tag="ag1_out")
                nc.sync.dma_start(ag1_in[0:1, :], s0row[:, :])
                nc.sync.dma_start(ag1_in[1:2, :], u0p[:, :])
                nc.gpsimd.collective_compute(
                    "AllGather", OP.bypass, replica_groups=RG,
                    ins=[ag1_in.opt()], outs=[ag1_out.opt()])
                agsb = ap.tile([16, 512], f32, tag="agsb")
                nc.sync.dma_start(agsb[:, :], ag1_out[:, :])
                # transpose to column layout: s0u0f[p, c*16 + r*2 + j]
                # (stage-B chunk kc = c*8 + r covers s0 indices r*512+c*128+p)
                s0u0f = ap.tile([128, 64], f32, tag="s0u0f")
                for c in range(4):
                    ptr = pp.tile([128, 16], f32, tag="lat", name=f"ptr{c}")
                    nc.tensor.transpose(
                        ptr[:, :], agsb[:, c * 128:(c + 1) * 128], i16[:, :])
                    nc.vector.tensor_copy(
                        s0u0f[:, c * 16:(c + 1) * 16], ptr[:, :])

                # ------- stage B: a1 shard = W1s(128 rows) @ [s0 u0'] -------
                pB = pp.tile([128, 2], f32, tag="big")
                for kc in range(32):
                    nc.tensor.matmul(
                        pB[:, :], lhsT=w1s[:, kc * 128:(kc + 1) * 128],
                        rhs=s0u0f[:, kc * 2:kc * 2 + 2],
                        start=(kc == 0), stop=(kc == 31))
                s1u1sh = ap.tile([128, 2], f32, tag="s1u1sh")
                tmps = ap.tile([128, 1], f32, tag="tmps")
                nc.vector.tensor_tensor(
                    out=tmps[:, :], in0=pB[:, 0:1], in1=b1sh[:, :], op=OP.add)
                nc.scalar.activation(s1u1sh[:, 0:1], tmps[:, :], AF.Sigmoid)
                d1s = ap.tile([128, 1], f32, tag="d1s")
                nc.vector.tensor_scalar(
                    out=d1s[:, :], in0=s1u1sh[:, 0:1], scalar1=-1.0,
                    scalar2=1.0, op0=OP.mult, op1=OP.add)
                nc.vector.tensor_tensor(
                    out=d1s[:, :], in0=d1s[:, :], in1=s1u1sh[:, 0:1],
                    op=OP.mult)
                nc.vector.tensor_tensor(
                    out=s1u1sh[:, 1:2], in0=d1s[:, :], in1=pB[:, 1:2],
                    op=OP.mult)

                # ------- AllGather 2: full [s1; u1'] ----------
                ag2_in = dp.tile([128, 2], f32, tag="ag2_in")
                ag2_out = dp.tile([1024, 2], f32, tag="ag2_out")
                nc.sync.dma_start(ag2_in[:, :], s1u1sh[:, :])
                nc.gpsimd.collective_compute(
                    "AllGather", OP.bypass, replica_groups=RG,
                    ins=[ag2_in.opt()], outs=[ag2_out.opt()])
                s1u1f = ap.tile([128, 16], f32, tag="s1u1f")
                nc.sync.dma_start(
                    s1u1f[:, :],
                    ag2_out.rearrange("(r p) j -> p r j", p=128))

                # ------- stage C: [z_pre dz] = W2 @ [s1 u1'] ----------
                pc = pp.tile([16, 2], f32, tag="lat")
                for kc in range(8):
                    nc.tensor.matmul(
                        pc[:, :], lhsT=w2s[:, kc * 16:(kc + 1) * 16],
                        rhs=s1u1f[:, kc * 2:kc * 2 + 2],
                        start=(kc == 0), stop=(kc == 7))
                lat = ap.tile([16, 4], f32, tag="lat_sb")
                nc.vector.tensor_tensor(
                    out=lat[:, 0:1], in0=pc[:, 0:1], in1=b2c[:, :], op=OP.add)
                nc.vector.tensor_copy(lat[:, 1:2], pc[:, 1:2])
                zcol = lat[:, 0:1]

                # ------- SINDy latent: dzb = E_w @ theta(z) + E_b ----------
                p_zr = pp.tile([1, 16], f32, tag="lat")
                nc.tensor.matmul(p_zr[:, :], lhsT=zcol, rhs=i16[:, :],
                                 start=True, stop=True)
                zrow = ap.tile([1, 16], f32, tag="zrow")
                nc.vector.tensor_copy(zrow[:, :], p_zr[:, :])
                p_zz = pp.tile([16, 16], f32, tag="lat")
                nc.tensor.matmul(p_zz[:, :], lhsT=zrow[:, :], rhs=zrow[:, :],
                                 start=True, stop=True)
                zz = ap.tile([16, 16], f32, tag="zz")
                nc.vector.tensor_copy(zz[:, :], p_zz[:, :])
                p_a2 = pp.tile([128, 32], f32, tag="lat")
                nc.tensor.matmul(p_a2[:, 0:16], lhsT=r2a[:, :], rhs=zz[:, :],
                                 start=True, stop=False)
                nc.tensor.matmul(p_a2[:, 16:32], lhsT=r2b[:, :], rhs=zz[:, :],
                                 start=False, stop=True)
                a2 = ap.tile([128, 32], f32, tag="a2")
                nc.vector.tensor_copy(a2[:, :], p_a2[:, :])
                p_zp = pp.tile([128, 1], f32, tag="lat")
                nc.tensor.matmul(p_zp[:, :], lhsT=r1[:, :], rhs=zcol,
                                 start=True, stop=True)
                zrep = ap.tile([128, 1], f32, tag="zrep")
                nc.vector.tensor_copy(zrep[:, :], p_zp[:, :])
                p3v = ap.tile([128, 32], f32, tag="p3v")
                nc.vector.tensor_scalar(
                    out=p3v[:, :], in0=a2[:, :], scalar1=zrep[:, 0:1],
                    scalar2=None, op0=OP.mult)
                p_dzb = pp.tile([16, 1], f32, tag="lat")
                nc.tensor.matmul(p_dzb[:, :], lhsT=ewl[:, :], rhs=zcol,
                                 start=True, stop=False)
                for b in range(16):
                    nc.tensor.matmul(
                        p_dzb[:, :], lhsT=c2s[:, b * 16:(b + 1) * 16],
                        rhs=zz[:, b:b + 1], start=False, stop=False)
                for f in range(32):
                    nc.tensor.matmul(
                        p_dzb[:, :], lhsT=c3s[:, f * 16:(f + 1) * 16],
                        rhs=p3v[:, f:f + 1], start=False, stop=(f == 31))
                nc.vector.tensor_tensor(
                    out=lat[:, 2:3], in0=p_dzb[:, :], in1=c0s[:, :], op=OP.add)

                # ------- stage L0: [t0_pre v0] = wd0 @ [z dzb] ----------
                zdzb = ap.tile([16, 2], f32, tag="zdzb")
                nc.vector.tensor_copy(zdzb[:, 0:1], lat[:, 0:1])
                nc.vector.tensor_copy(zdzb[:, 1:2], lat[:, 2:3])
                pl0 = pp.tile([128, 16], f32, tag="big")
                pl0_r = pview(pl0)
                for mt in range(8):
                    nc.tensor.matmul(
                        pl0[:, mt * 2:(mt + 1) * 2],
                        lhsT=wd0s[:, mt * 128:(mt + 1) * 128], rhs=zdzb[:, :],
                        start=(mt == 0), stop=(mt == 7))
                t0v0 = ap.tile([128, 16], f32, tag="t0v0")
                tmp2 = ap.tile([128, 8], f32, tag="tmp2")
                nc.vector.tensor_tensor(
                    out=tmp2[:, :], in0=pl0_r[:, :, 0], in1=bd0f[:, :],
                    op=OP.add)
                nc.scalar.activation(t0v0[:, 0:8], tmp2[:, :], AF.Sigmoid)
                d2 = ap.tile([128, 8], f32, tag="d2")
                nc.vector.tensor_scalar(
                    out=d2[:, :], in0=t0v0[:, 0:8], scalar1=-1.0, scalar2=1.0,
                    op0=OP.mult, op1=OP.add)
                nc.vector.tensor_tensor(
                    out=d2[:, :], in0=d2[:, :], in1=t0v0[:, 0:8], op=OP.mult)
                nc.vector.tensor_tensor(
                    out=t0v0[:, 8:16], in0=d2[:, :], in1=pl0_r[:, :, 1],
                    op=OP.mult)
                t0v0_r = t0v0.rearrange("p (j c) -> p c j", j=2)

                # ------- stage L1: wd1 shard @ [t0 v0'] (512 rows/core) -----
                pl1 = pp.tile([128, 8], f32, tag="big")
                pl1_r = pview(pl1)
                for kc in range(8):
                    rhs = t0v0_r[:, kc, :]
                    for mt in range(4):
                        nc.tensor.matmul(
                            pl1[:, mt * 2:(mt + 1) * 2],
                            lhsT=wd1s[:, kc * 512 + mt * 128:
                                      kc * 512 + (mt + 1) * 128],
                            rhs=rhs,
                            start=(kc == 0 and mt == 0),
                            stop=(kc == 7 and mt == 3))
                tv = ap.tile([128, 8], f32, tag="tv")
                tmp3 = ap.tile([128, 4], f32, tag="tmp3")
                nc.vector.tensor_tensor(
                    out=tmp3[:, :], in0=pl1_r[:, :, 0], in1=bd1f[:, :],
                    op=OP.add)
                nc.scalar.activation(tv[:, 0:4], tmp3[:, :], AF.Sigmoid)
                d3 = ap.tile([128, 4], f32, tag="d3")
                nc.vector.tensor_scalar(
                    out=d3[:, :], in0=tv[:, 0:4], scalar1=-1.0, scalar2=1.0,
                    op0=OP.mult, op1=OP.add)
                nc.vector.tensor_tensor(
                    out=d3[:, :], in0=d3[:, :], in1=tv[:, 0:4], op=OP.mult)
                nc.vector.tensor_tensor(
                    out=tv[:, 4:8], in0=d3[:, :], in1=pl1_r[:, :, 1],
                    op=OP.mult)

                # ------- AllGather 3: full [t1 v1'] ----------
                ag3_in = dp.tile([128, 8], f32, tag="ag3_in")
                ag3_out = dp.tile([1024, 8], f32, tag="ag3_out")
                nc.sync.dma_start(ag3_in[:, :], tv[:, :])
                nc.gpsimd.collective_compute(
                    "AllGather", OP.bypass, replica_groups=RG,
                    ins=[ag3_in.opt()], outs=[ag3_out.opt()])
                t1full = cp.tile([128, 64], f32, tag="t1full")
                nc.sync.dma_start(
                    t1full[:, :],
                    ag3_out.rearrange("(r p) q -> p r q", p=128))

            # split [t1 v1'] into bf16 hi+lo, laid out per k-chunk:
            # t1hl[p, kc*4 + h*2 + j], kc = r*4 + m
            t1v = t1full.rearrange("p (r j m) -> p r m j", r=8, j=2, m=4)
            t1hl = cp.tile([128, 128], bf16, tag="t1hl")
            t1hl_v = t1hl.rearrange("p (r m h j) -> p h r m j",
                                    r=8, m=4, h=2, j=2)
            nc.vector.tensor_copy(t1hl_v[:, 0], t1v)
            hif32 = ap.tile([128, 64], f32, tag="hif32")
            hif32_v = hif32.rearrange("p (r m j) -> p r m j", r=8, m=4, j=2)
            nc.vector.tensor_copy(hif32_v, t1hl_v[:, 0])
            lof32 = ap.tile([128, 64], f32, tag="lof32")
            lof32_v = lof32.rearrange("p (r m j) -> p r m j", r=8, m=4, j=2)
            nc.vector.tensor_tensor(
                out=lof32_v, in0=t1v, in1=hif32_v, op=OP.subtract)
            nc.vector.tensor_copy(t1hl_v[:, 1], lof32_v)

            with tc.tile_pool(name="ps2", bufs=1, space="PSUM") as pp2:
                # ------- stage E: [xb dxb] = wd2 shard @ [t1 v1'] ----------
                # 2 MMs per (chunk, block): [hi lo]x4 @ Whi and hi x Wlo;
                # the hi*Whi + lo*Whi + hi*Wlo sum is folded afterwards by
                # two tiny combine matmuls per block.
                pE4 = [pp2.tile([4, 512], f32, tag=f"e4{nb}", name=f"pE4{nb}")
                       for nb in range(4)]
                pE2 = [pp2.tile([2, 512], f32, tag=f"e2{nb}", name=f"pE2{nb}")
                       for nb in range(4)]
                for kc in range(32):
                    slab = wd2p.tile([128, 4096], bf16, tag="wd2slab")
                    nc.sync.dma_start(slab[:, :], d_wd2[kc])
                    hl4 = t1hl[:, kc * 4:kc * 4 + 4]
                    hi = t1hl[:, kc * 4:kc * 4 + 2]
                    for nb in range(4):
                        whi = slab[:, nb * 512:(nb + 1) * 512]
                        wlo = slab[:, 2048 + nb * 512:2048 + (nb + 1) * 512]
                        nc.tensor.matmul(
                            pE4[nb][:, :], lhsT=hl4, rhs=whi,
                            start=(kc == 0), stop=(kc == 31))
                        nc.tensor.matmul(
                            pE2[nb][:, :], lhsT=hi, rhs=wlo,
                            start=(kc == 0), stop=(kc == 31))

                for nb in range(4):
                    a4 = ap.tile([4, 512], f32, tag="a4", bufs=2,
                                 name=f"a4{nb}")
                    nc.vector.tensor_copy(a4[:, :], pE4[nb][:, :])
                    a2 = ap.tile([2, 512], f32, tag="a2e", bufs=2,
                                 name=f"a2e{nb}")
                    nc.vector.tensor_copy(a2[:, :], pE2[nb][:, :])
                    pC = pp2.tile([2, 512], f32, tag=f"e4{nb}",
                                  name=f"pC{nb}")
                    nc.tensor.matmul(pC[:, :], lhsT=c4c[:, :], rhs=a4[:, :],
                                     start=True, stop=False)
                    nc.tensor.matmul(pC[:, :], lhsT=i16[0:2, 0:2],
                                     rhs=a2[:, :], start=False, stop=True)
                    erows = ap.tile([2, 512], f32, tag="erows", bufs=2,
                                    name=f"erows{nb}")
                    nc.vector.tensor_copy(erows[:, :], pC[:, :])
                    xst = ap.tile([1, 512], f32, tag="xst", bufs=2,
                                  name=f"xst{nb}")
                    nc.vector.tensor_tensor(
                        out=xst[:, :], in0=erows[0:1, :],
                        in1=bd2row[:, nb * 512:(nb + 1) * 512], op=OP.add)
                    nc.sync.dma_start(
                        d_xb.ap()[nb * 512:(nb + 1) * 512], xst[0:1, :])
                    nc.sync.dma_start(
                        d_dxb.ap()[nb * 512:(nb + 1) * 512], erows[1:2, :])
                nc.sync.dma_start(d_lat[:, :], lat[:, 0:3])

    nc.compile()
    return nc


def get_program():
    if "nc" not in _CACHE:
        _CACHE["nc"] = _build_program()
    return _CACHE["nc"]


# ------------------------------------------------------------- host prep

def _split_bf16(a):
    hi = a.astype(BF16)
    lo = (a - hi.astype(np.float32)).astype(BF16)
    return hi, lo


def prepare_in_maps(inputs):
    """Shard + lay out the full inputs into 8 per-core input maps."""
    f = np.float32
    x = np.asarray(inputs["x"], f)
    dx = np.asarray(inputs["dx"], f)
    E_w = np.asarray(inputs["E_w"], f)
    E_b = np.asarray(inputs["E_b"], f)
    we_w0 = np.asarray(inputs["we_w0"], f)
    we_b0 = np.asarray(inputs["we_b0"], f)
    we_w1 = np.asarray(inputs["we_w1"], f)
    we_b1 = np.asarray(inputs["we_b1"], f)
    we_w2 = np.asarray(inputs["we_w2"], f)
    we_b2 = np.asarray(inputs["we_b2"], f)
    wd_w0 = np.asarray(inputs["wd_w0"], f)
    wd_b0 = np.asarray(inputs["wd_b0"], f)
    wd_w1 = np.asarray(inputs["wd_w1"], f)
    wd_b1 = np.asarray(inputs["wd_b1"], f)
    wd_w2 = np.asarray(inputs["wd_w2"], f)
    wd_b2 = np.asarray(inputs["wd_b2"], f)

    # vins[p, kc*4 + h*2 + j]: bf16 hi/lo split of x, dx at index kc*128+p
    xhi, xlo = _split_bf16(x)
    dxhi, dxlo = _split_bf16(dx)
    vins = np.empty((128, 128, 2, 2), BF16)       # [p, kc, h, j]
    vins[:, :, 0, 0] = xhi.reshape(128, 128).T
    vins[:, :, 0, 1] = dxhi.reshape(128, 128).T
    vins[:, :, 1, 0] = xlo.reshape(128, 128).T
    vins[:, :, 1, 1] = dxlo.reshape(128, 128).T
    vins = np.ascontiguousarray(vins.reshape(128, 512))

    # SINDy coefficient folding (replicated)
    c0 = (E_b + E_w[:, 0:L].sum(axis=1)).reshape(16, 1).astype(f)
    ewlint = np.ascontiguousarray(E_w[:, L:2 * L].T)
    c2stack = np.zeros((16, 256), f)
    q = 2 * L
    for i in range(L):
        for j in range(i, L):
            c2stack[i, j * 16:(j + 1) * 16] = E_w[:, q]
            q += 1
    c3stack = np.zeros((128, 512), f)
    for i in range(L):
        for j in range(i, L):
            for k in range(j, L):
                p = i * 8 + j // 2
                fidx = (j % 2) * 16 + k
                c3stack[p, fidx * 16:(fidx + 1) * 16] += E_w[:, q]
                q += 1
    assert q == E_w.shape[1]

    r1 = np.zeros((16, 128), f)
    r1[np.arange(128) // 8, np.arange(128)] = 1.0
    r2a = np.zeros((16, 128), f)
    r2a[2 * (np.arange(128) % 8), np.arange(128)] = 1.0
    r2b = np.zeros((16, 128), f)
    r2b[2 * (np.arange(128) % 8) + 1, np.arange(128)] = 1.0

    shared = {
        "vins": vins,
        "w2t": np.ascontiguousarray(
            we_w2.T.reshape(8, 128, 16).transpose(1, 0, 2).reshape(128, 128)),
        "wd0t": np.ascontiguousarray(wd_w0.T),
        "b2c": we_b2.reshape(16, 1),
        "bd0f": np.ascontiguousarray(wd_b0.reshape(8, 128).T),
        "i16": np.eye(16, dtype=f),
        "r1": r1, "r2a": r2a, "r2b": r2b,
        "c3stack": c3stack, "c2stack": c2stack,
        "ewlint": ewlint, "c0": c0,
        "c4comb": np.array([[1, 0], [0, 1], [1, 0], [0, 1]], f),
    }

    in_maps = []
    for c in range(NCORES):
        r0 = slice(c * H1S, (c + 1) * H1S)        # w0 rows / wd1 rows
        r1b = slice(c * H2S, (c + 1) * H2S)       # w1 rows (128)
        r2 = slice(c * OUTS, (c + 1) * OUTS)      # wd2 rows
        m = dict(shared)

        w0T = np.ascontiguousarray(we_w0[r0, :].T)          # [16384, 512]
        w0hi, w0lo = _split_bf16(w0T)
        # slab layout [g, p, (k, h, m)] so each partition row is contiguous
        w0 = np.empty((32, 4, 2, 128, 512), BF16)           # [g, k, h, p, m]
        w0[:, :, 0] = w0hi.reshape(32, 4, 128, 512)
        w0[:, :, 1] = w0lo.reshape(32, 4, 128, 512)
        m["w0t"] = np.ascontiguousarray(
            w0.transpose(0, 3, 1, 2, 4).reshape(32, 128, 4096))

        w1T = we_w1[r1b, :].T.reshape(8, 4, 128, 128)      # [r, c, p, m]
        m["w1t"] = np.ascontiguousarray(
            w1T.transpose(2, 1, 0, 3).reshape(128, 4096))
        m["wd1t"] = np.ascontiguousarray(
            wd_w1[r0, :].T.reshape(8, 128, 512).transpose(1, 0, 2).reshape(
                128, 4096))

        wd2T = np.ascontiguousarray(wd_w2[r2, :].T)         # [4096, 2048]
        w2hi, w2lo = _split_bf16(wd2T)
        wd2 = np.empty((32, 2, 128, 2048), BF16)            # [kc, h, p, m]
        wd2[:, 0] = w2hi.reshape(32, 128, 2048)
        wd2[:, 1] = w2lo.reshape(32, 128, 2048)
        m["wd2t"] = np.ascontiguousarray(
            wd2.transpose(0, 2, 1, 3).reshape(32, 128, 4096))

        m["b0row"] = we_b0[r0].reshape(1, 512)
        m["b1sh"] = we_b1[r1b].reshape(128, 1)
        m["bd1f"] = np.ascontiguousarray(wd_b1[r0].reshape(4, 128).T)
        m["bd2row"] = wd_b2[r2].reshape(1, 2048)
        in_maps.append(m)
    return in_maps


def assemble_outputs(results):
    """results: list (per core) of {name: np.ndarray}."""
    lat = np.asarray(results[0]["lat"])
    z = np.ascontiguousarray(lat[:, 0])
    dz = np.ascontiguousarray(lat[:, 1])
    dzb = np.ascontiguousarray(lat[:, 2])
    xb = np.concatenate(
        [np.asarray(results[c]["xb_sh"]).reshape(-1) for c in range(NCORES)])
    dxb = np.concatenate(
        [np.asarray(results[c]["dxb_sh"]).reshape(-1) for c in range(NCORES)])
    return (z, dz, dzb, xb, dxb)


def kernel(**inputs):
    from concourse.bass_utils import run_bass_kernel_spmd
    nc = get_program()
    in_maps = prepare_in_maps(inputs)
    res = run_bass_kernel_spmd(nc, in_maps, core_ids=list(range(NCORES)))
    return assemble_outputs(res.results)
